# revision 1
# baseline (speedup 1.0000x reference)
"""Trainium2 Bass kernel for nn_KnowledgeBaseLookup (bucketed dma_gather design).

Computation (see reference):
    lookup = knowledge_base[indexes]            # (B,T,K,D) gather
    y      = einsum('btk,btkd->btd', weights, lookup)
    out    = y @ w_out.T + b_out                # (B,T,E)

Sharding: data-parallel over the B*T token dim across 8 cores; the
knowledge_base table is replicated per core.

Per-core design (1024 tokens, 16384 gathered rows):
  The old per-slab indirect-DMA gather paid a ~1us SWDGE desc-gen fixed cost
  per 128 rows (128 Pool instructions -> Pool-bound at ~140us).  Instead we
  use the batched `dma_gather` custom op (one instruction per 1024 rows), at
  the price of int16 indices: indices are bucketed by table chunk of 32768
  rows so chunk-local indices fit in int16, with the chunk base carried by
  the in_ap view.

  Layout: tokens split into 2 halves of 512; each half into 8 subgroups of
  64 tokens.  For each (half h, chunk b) one dma_gather call fetches 1024
  rows = 8 slabs of 128 slots; slab j holds up to 128 (token,k) pairs of
  subgroup j whose table row lies in chunk b (capacity = the mean occupancy,
  128; the final bucket is issued as two 512-row calls into separate tiles
  so the tail epilogue starts a transfer earlier).  Overflow pairs go to a
  per-half spill region of 3 slabs gathered by classic indirect DMA (any
  chunk, int32 indices); spill matmuls run before (h1) / after (h0) the
  bucket matmuls to match when their data lands.

  Reduction: for each slab, a [128,64] fp32r mask M[slot, j] =
  w[slot] * (tokloc[slot] == j) is built on DVE (is_equal on an iota table,
  then multiply; tokloc/weights are host-prepped per slot).  PE matmuls
  lhsT=rows (fp32r, a free bitcast of the gathered fp32) x rhs=mask
  accumulate yT[d, token] into per-group-pair PSUM banks (zeroed once via
  memset; start=True would wipe the whole 2KB zero-region).  The spill
  slabs use a 512-wide mask over the whole half.  Stage 2 (out_proj)
  contracts yT (bf16) with w_out.T (bf16) per 128-token group, folds the
  bias in as a K=1 matmul, copies PSUM->SBUF on the idle Activation
  engine, and DMAs out.

  The dma_gather Q7 ucode reads index i of a call from the idx tile at
  [16 + i%16, i//16] on the NEFF path (queue 0 channel base), while the
  bass-level interpreter reads [i%16, i//16]; the host writes both bands.
"""

import numpy as np

B, T, K = 4, 2048, 16
C, D, E = 262144, 256, 512
NCORES = 8
NTOK = B * T                      # 8192 tokens
TPC = NTOK // NCORES              # 1024 tokens per core
P = 128
HALVES = 2
HTOK = TPC // HALVES              # 512 tokens per half
NB = 8                            # value chunks
CHUNK = C // NB                   # 32768 rows, int16-addressable
NW = 8                            # subgroups per half
WTOK = HTOK // NW                 # 64 tokens per subgroup
NIDX_CALL = NW * P                # 1024 indices per dma_gather call
SPILL_SLABS = 3                   # per half
SPILL_CAP = SPILL_SLABS * P       # 512
MAIN_SLABS = HALVES * NB * NW     # 128
SPILL_TOT = HALVES * SPILL_SLABS  # 8

_CACHE = {}


def _build_bass():
    import concourse.bass as bass
    import concourse.mybir as mybir
    from concourse import bacc, library_config
    from concourse.tile import TileContext

    fp32 = mybir.dt.float32
    f32r = mybir.dt.float32r
    bf16 = mybir.dt.bfloat16
    i16 = mybir.dt.int16
    i32 = mybir.dt.int32
    eq = mybir.AluOpType.is_equal
    mul = mybir.AluOpType.mult
    nc = bacc.Bacc(
        "TRN2", target_bir_lowering=False, debug=False, num_devices=NCORES
    )

    kb = nc.dram_tensor("kb", [C, D], f32r, kind="ExternalInput")
    idx16 = nc.dram_tensor("idx16", [P, HALVES * NB * (NIDX_CALL // 16)], i16,
                           kind="ExternalInput")
    idxsp = nc.dram_tensor("idxsp", [P, SPILL_TOT], i32, kind="ExternalInput")
    wslot = nc.dram_tensor("wslot", [P, MAIN_SLABS], fp32, kind="ExternalInput")
    tokloc = nc.dram_tensor("tokloc", [P, MAIN_SLABS], i16, kind="ExternalInput")
    wsp = nc.dram_tensor("wsp", [P, SPILL_TOT], fp32, kind="ExternalInput")
    toksp = nc.dram_tensor("toksp", [P, SPILL_TOT], i16, kind="ExternalInput")
    iota64 = nc.dram_tensor("iota64", [P, WTOK], i16, kind="ExternalInput")
    wout = nc.dram_tensor("wout", [P, 2 * E], bf16, kind="ExternalInput")
    bias = nc.dram_tensor("bias", [1, E], bf16, kind="ExternalInput")
    ones = nc.dram_tensor("ones", [1, P], bf16, kind="ExternalInput")
    out = nc.dram_tensor("out", [TPC, E], fp32, kind="ExternalOutput")

    COLS = NIDX_CALL // 16  # idx16 columns per call

    with TileContext(nc) as tc:
        with (
            tc.tile_pool(name="const", bufs=1) as cpool,
            tc.tile_pool(name="gath", bufs=10) as gpool,
            tc.tile_pool(name="gtail", bufs=1) as gtpool,
            tc.tile_pool(name="mask", bufs=4) as mpool,
            tc.tile_pool(name="spill", bufs=2) as sppool,
            tc.tile_pool(name="spmask", bufs=2) as smpool,
            tc.tile_pool(name="y", bufs=2) as ypool,
            tc.tile_pool(name="o", bufs=8) as opool,
            tc.tile_pool(name="psy", bufs=2, space="PSUM") as psy,
            tc.tile_pool(name="pso", bufs=2, space="PSUM") as pso,
        ):
            idx_sb = cpool.tile([P, HALVES * NB * COLS], i16)
            nc.gpsimd.dma_start(out=idx_sb[:], in_=idx16[:, :])
            nc.gpsimd.load_library(library_config.mlp)
            idxsp_sb = cpool.tile([P, SPILL_TOT], i32)
            nc.sync.dma_start(out=idxsp_sb[:], in_=idxsp[:, :])
            w_sb = cpool.tile([P, MAIN_SLABS], fp32)
            nc.sync.dma_start(out=w_sb[:], in_=wslot[:, :])
            tl_sb = cpool.tile([P, MAIN_SLABS], i16)
            nc.sync.dma_start(out=tl_sb[:], in_=tokloc[:, :])
            wsp_sb = cpool.tile([P, SPILL_TOT], fp32)
            nc.sync.dma_start(out=wsp_sb[:], in_=wsp[:, :])
            tsp_sb = cpool.tile([P, SPILL_TOT], i16)
            nc.sync.dma_start(out=tsp_sb[:], in_=toksp[:, :])
            io64_sb = cpool.tile([P, WTOK], i16)
            nc.sync.dma_start(out=io64_sb[:], in_=iota64[:, :])
            io512_sb = cpool.tile([P, HTOK], i16)
            wo_sb = cpool.tile([P, 2 * E], bf16)
            nc.sync.dma_start(out=wo_sb[:], in_=wout[:, :])
            b_sb = cpool.tile([1, E], bf16)
            nc.sync.dma_start(out=b_sb[:], in_=bias[:, :])
            one_sb = cpool.tile([1, P], bf16)
            nc.sync.dma_start(out=one_sb[:], in_=ones[:, :])

            for h in range(HALVES):
                # one psum tile per PAIR of 128-token groups (bank-sized) so
                # each pair's epilogue depends only on its own writers
                # (start=True zeroes the whole 2KB psum zero-region, so zero
                # once and accumulate-only).  col = g_loc*2P + ch*P + window.
                ytp = []
                for pr in range(HTOK // P // 2):
                    t_ = psy.tile([P, 4 * P], fp32, tag=f"yt{pr}")
                    nc.vector.memset(t_[:], 0.0)
                    ytp.append(t_)

                gs = []
                for b in range(NB):
                    col0 = (h * NB + b) * COLS
                    if h == 1 and b == NB - 1:
                        # last bucket: two 512-idx calls into SEPARATE tiles
                        # (deps are tile-granular) so pair 0's epilogue starts
                        # one transfer earlier
                        ga = gtpool.tile([P, NW // 2, D], f32r, tag="ga")
                        nc.gpsimd.dma_gather(
                            out_ap=ga[:],
                            in_ap=kb[b * CHUNK:(b + 1) * CHUNK, :],
                            idxs_ap=idx_sb[:, col0:col0 + COLS // 2],
                            num_idxs=NIDX_CALL // 2,
                            num_idxs_reg=NIDX_CALL // 2,
                            elem_size=D,
                        )
                        gb = gtpool.tile([P, NW // 2, D], f32r, tag="gb")
                        nc.gpsimd.dma_gather(
                            out_ap=gb[:],
                            in_ap=kb[b * CHUNK:(b + 1) * CHUNK, :],
                            idxs_ap=idx_sb[:, col0 + COLS // 2:col0 + COLS],
                            num_idxs=NIDX_CALL // 2,
                            num_idxs_reg=NIDX_CALL // 2,
                            elem_size=D,
                        )
                        gs.append((ga, gb))
                    else:
                        g = gpool.tile([P, NW, D], f32r, tag="g")
                        nc.gpsimd.dma_gather(
                            out_ap=g[:],
                            in_ap=kb[b * CHUNK:(b + 1) * CHUNK, :],
                            idxs_ap=idx_sb[:, col0:col0 + COLS],
                            num_idxs=NIDX_CALL,
                            num_idxs_reg=NIDX_CALL,
                            elem_size=D,
                        )
                        gs.append(g)

                if h == 0:
                    # issue BOTH halves' spill gathers now: their desc-gen
                    # overlaps h0's transfers and the data arrives well before
                    # each half's epilogue (instead of queueing after all
                    # gathers and serializing the tail).
                    sp_tiles = []
                    for hh in range(HALVES):
                        sp = sppool.tile([P, SPILL_SLABS, D], f32r, tag="sp")
                        for s in range(SPILL_SLABS):
                            col = hh * SPILL_SLABS + s
                            nc.gpsimd.indirect_dma_start(
                                out=sp[:, s, :],
                                out_offset=None,
                                in_=kb[:, :],
                                in_offset=bass.IndirectOffsetOnAxis(
                                    ap=idxsp_sb[:, col:col + 1], axis=0
                                ),
                            )
                        sp_tiles.append(sp)
                    nc.gpsimd.iota(io512_sb[:], [[1, HTOK]],
                                   channel_multiplier=0)
                sp = sp_tiles[h]

                def do_spill(last):
                    msp = smpool.tile([P, SPILL_SLABS, HTOK], f32r, tag="msp")
                    sblk = h * SPILL_SLABS
                    nc.vector.tensor_tensor(
                        out=msp[:],
                        in0=io512_sb[:].unsqueeze(1)
                            .broadcast_to([P, SPILL_SLABS, HTOK]),
                        in1=tsp_sb[:, sblk:sblk + SPILL_SLABS].unsqueeze(2)
                            .broadcast_to([P, SPILL_SLABS, HTOK]),
                        op=eq,
                    )
                    nc.vector.tensor_tensor(
                        out=msp[:],
                        in0=msp[:],
                        in1=wsp_sb[:, sblk:sblk + SPILL_SLABS].unsqueeze(2)
                            .broadcast_to([P, SPILL_SLABS, HTOK]),
                        op=mul,
                    )
                    for s in range(SPILL_SLABS):
                        for ch in range(2):
                            for pr in range(HTOK // P // 2):
                                # 256-token pair slice; out cols (g_loc, w)
                                # with stride 2P match rhs (g_loc, w) stride P
                                nc.tensor.matmul(
                                    out=ytp[pr][:].rearrange(
                                        "p (g c w) -> p g c w", g=2, c=2
                                    )[:, :, ch, :],
                                    lhsT=sp[:, s, ch * P:(ch + 1) * P],
                                    rhs=msp[:, s, pr * 2 * P:(pr + 1) * 2 * P]
                                        .rearrange("p (g w) -> p g w", g=2),
                                    start=False,
                                    stop=(last and s == SPILL_SLABS - 1),
                                    skip_group_check=True,
                                )

                if h == 1:
                    # h1 spill data lands long before its last gathers:
                    # run its matmuls first so only bucket 7 is in the tail.
                    do_spill(last=False)

                # mask-matmul reduction, bucket by bucket
                for b in range(NB):
                    blk = (h * NB + b) * NW
                    mask = mpool.tile([P, NW, WTOK], f32r, tag="m")
                    nc.vector.tensor_tensor(
                        out=mask[:],
                        in0=io64_sb[:].unsqueeze(1).broadcast_to([P, NW, WTOK]),
                        in1=tl_sb[:, blk:blk + NW].unsqueeze(2)
                            .broadcast_to([P, NW, WTOK]),
                        op=eq,
                    )
                    nc.vector.tensor_tensor(
                        out=mask[:],
                        in0=mask[:],
                        in1=w_sb[:, blk:blk + NW].unsqueeze(2)
                            .broadcast_to([P, NW, WTOK]),
                        op=mul,
                    )
                    for j in range(NW):
                        for ch in range(2):
                            nc.tensor.matmul(
                                out=ytp[j // 4][:, (j // 2 % 2) * 2 * P + ch * P
                                    + (j % 2) * WTOK:(j // 2 % 2) * 2 * P
                                    + ch * P + (j % 2 + 1) * WTOK],
                                lhsT=(gs[b][j // 4][:, j % 4, ch * P:(ch + 1) * P]
                                      if isinstance(gs[b], tuple) else
                                      gs[b][:, j, ch * P:(ch + 1) * P]),
                                rhs=mask[:, j, :],
                                start=False,
                                stop=(h == 1 and b == NB - 1 and j >= NW - 2),
                                skip_group_check=True,
                            )

                if h == 0:
                    do_spill(last=True)

                yb = ypool.tile([P, 2 * HTOK], bf16, tag="yb")
                for pr in range(HTOK // P // 2):
                    nc.vector.tensor_copy(
                        out=yb[:, pr * 4 * P:(pr + 1) * 4 * P],
                        in_=ytp[pr][:],
                    )

                for g4 in range(HTOK // P):
                    ops = pso.tile([P, E], fp32, tag="ops")
                    for ch in range(2):
                        nc.tensor.matmul(
                            out=ops[:],
                            lhsT=yb[:, (g4 // 2) * 4 * P + (g4 % 2) * 2 * P
                                    + ch * P:(g4 // 2) * 4 * P
                                    + (g4 % 2) * 2 * P + (ch + 1) * P],
                            rhs=wo_sb[:, ch * E:(ch + 1) * E],
                            start=(ch == 0),
                            stop=False,
                        )
                    # bias add as a K=1 matmul: ones[1,P]^T x bias[1,E]
                    nc.tensor.matmul(
                        out=ops[:],
                        lhsT=one_sb[:, :],
                        rhs=b_sb[:, :],
                        start=False,
                        stop=True,
                    )
                    osb = opool.tile([P, E], fp32, tag="osb")
                    nc.scalar.copy(out=osb[:], in_=ops[:])
                    row0 = (h * (HTOK // P) + g4) * P
                    nc.sync.dma_start(out=out[row0:row0 + P, :], in_=osb[:])

    nc.compile()
    return nc


def _host_prep(weights, indexes, w_out, b_out):
    """Bucket/sort (token,k) pairs per core and build all device-side arrays."""
    wflat = np.ascontiguousarray(weights, dtype=np.float32).reshape(NTOK, K)
    iflat = np.ascontiguousarray(indexes).reshape(NTOK, K).astype(np.int64)

    import ml_dtypes
    woutT = np.ascontiguousarray(w_out, dtype=np.float32).T      # [D, E]
    wout_host = np.ascontiguousarray(
        woutT.reshape(2, P, E).transpose(1, 0, 2).reshape(P, 2 * E)
    ).astype(ml_dtypes.bfloat16)
    bias_host = np.asarray(b_out, dtype=np.float32).reshape(1, E).astype(ml_dtypes.bfloat16)
    ones_host = np.ones((1, P), dtype=ml_dtypes.bfloat16)
    iota64_h = np.ascontiguousarray(
        np.broadcast_to(np.arange(WTOK, dtype=np.int16), (P, WTOK))
    )

    COLS = NIDX_CALL // 16
    in_maps = []
    for c in range(NCORES):
        ic = iflat[c * TPC:(c + 1) * TPC].ravel()          # [16384]
        wc = wflat[c * TPC:(c + 1) * TPC].ravel()
        t = np.repeat(np.arange(TPC, dtype=np.int64), K)   # token per pair

        h = t // HTOK
        wsub = (t % HTOK) // WTOK
        b = ic // CHUNK
        key = (h * NB + b) * NW + wsub                     # 0..127 slab id

        order = np.argsort(key, kind="stable")
        ks = key[order]
        iv = ic[order]
        wv = wc[order]
        tv = t[order]
        starts = np.searchsorted(ks, np.arange(MAIN_SLABS))
        rank = np.arange(TPC * K) - starts[ks]

        idx16_host = np.zeros((P, HALVES * NB * COLS), np.int16)
        wslot_host = np.zeros((P, MAIN_SLABS), np.float32)
        tokloc_host = np.zeros((P, MAIN_SLABS), np.int16)
        idxsp_host = np.zeros((P, SPILL_TOT), np.int32)
        wsp_host = np.zeros((P, SPILL_TOT), np.float32)
        toksp_host = np.zeros((P, SPILL_TOT), np.int16)

        main = rank < P
        mk, mr = ks[main], rank[main]
        mi, mw, mt = iv[main], wv[main], tv[main]
        mh = mk // (NB * NW)
        mb = (mk // NW) % NB
        mj = mk % NW
        slot = mj * P + mr                                 # slot within call
        col = (mh * NB + mb) * COLS + slot // 16
        idx_local = (mi - mb * CHUNK).astype(np.int16)
        idx16_host[slot % 16, col] = idx_local             # interp layout
        idx16_host[16 + slot % 16, col] = idx_local        # NEFF Q7 layout
        wslot_host[mr, mk] = mw
        tokloc_host[mr, mk] = (mt - (mh * HTOK + mj * WTOK)).astype(np.int16)

        sh = ks[~main] // (NB * NW)                        # spill half
        si, sw, st = iv[~main], wv[~main], tv[~main]
        for hh in range(HALVES):
            sel = sh == hh
            n = int(sel.sum())
            if n > SPILL_CAP:
                raise ValueError(
                    f"spill overflow: core {c} half {hh} needs {n} > {SPILL_CAP}"
                )
            r = np.arange(n)
            idxsp_host[r % P, hh * SPILL_SLABS + r // P] = si[sel]
            wsp_host[r % P, hh * SPILL_SLABS + r // P] = sw[sel]
            toksp_host[r % P, hh * SPILL_SLABS + r // P] = (
                st[sel] - hh * HTOK
            ).astype(np.int16)

        in_maps.append({
            "idx16": idx16_host,
            "idxsp": idxsp_host,
            "wslot": wslot_host,
            "tokloc": tokloc_host,
            "wsp": wsp_host,
            "toksp": toksp_host,
            "iota64": iota64_h,
            "wout": wout_host,
            "bias": bias_host,
            "ones": ones_host,
        })
    return in_maps


def kernel(weights, indexes, knowledge_base, w_out, b_out):
    from concourse.bass_utils import run_bass_kernel_spmd

    if "nc" not in _CACHE:
        _CACHE["nc"] = _build_bass()
    nc = _CACHE["nc"]

    kb_host = np.ascontiguousarray(knowledge_base, dtype=np.float32)
    in_maps = _host_prep(weights, indexes, w_out, b_out)
    for m in in_maps:
        m["kb"] = kb_host

    res = run_bass_kernel_spmd(nc, in_maps, list(range(NCORES)))
    out = np.concatenate([res.results[c]["out"] for c in range(NCORES)], axis=0)
    return out.reshape(B, T, E).astype(np.float32)



# revision 11
# speedup vs baseline: 1.4794x; 1.4794x over previous
"""Trainium2 Bass kernel for nn_KnowledgeBaseLookup (bucketed dma_gather design).

Computation (see reference):
    lookup = knowledge_base[indexes]            # (B,T,K,D) gather
    y      = einsum('btk,btkd->btd', weights, lookup)
    out    = y @ w_out.T + b_out                # (B,T,E)

Sharding: data-parallel over the B*T token dim across 8 cores; the
knowledge_base table is replicated per core (converted to bf16 on host —
host prep also does the index bucketing/sorting).

Per-core design (1024 tokens, 16384 gathered rows), all-bf16 datapath:
  Rows are gathered with the batched `dma_gather` custom op (one instruction
  per 1024 rows) from a bf16 copy of the table, at the price of int16
  indices: indices are bucketed by table chunk of 32768 rows so chunk-local
  indices fit in int16, with the chunk base carried by the in_ap view.

  Layout: tokens split into 2 halves of 512; each half into 8 subgroups of
  64 tokens.  For each (half h, chunk b) one dma_gather call fetches 1024
  rows = 8 slabs of 128 slots; slab j holds up to 128 (token,k) pairs of
  subgroup j whose table row lies in chunk b (capacity = the mean occupancy,
  128; the final call is issued as four 256-row calls into separate tiles
  so the tail epilogue starts earlier).  Overflow pairs go to a per-half
  spill region of 3 slabs gathered by one multi-slab indirect DMA (any
  chunk, int32 indices); spill matmuls run right after bucket 0's (which
  carry start=True to zero the psum banks).

  Reduction: for each slab, a [128,64] bf16 mask M[slot, j] =
  w[slot] * (tokloc[slot] == j) is built on DVE (is_equal on an iota table,
  then multiply; tokloc/weights are host-prepped per slot).  PE matmuls
  lhsT=rows (bf16) x rhs=mask accumulate yT[d, token] into per-group-pair
  PSUM banks.  The spill slabs use a 512-wide mask over the whole half.
  Stage 2 (out_proj) contracts yT (bf16) with w_out.T (bf16) per 128-token
  group (bias is added on host), copies PSUM->SBUF in bf16 on the idle
  Activation engine, and DMAs out; the host upcasts to fp32.

  The dma_gather Q7 ucode reads index i of a call from the idx tile at
  [16 + i%16, i//16] on the NEFF path (queue 0 channel base), while the
  bass-level interpreter reads [i%16, i//16]; the host writes both bands.
"""

import numpy as np

B, T, K = 4, 2048, 16
C, D, E = 262144, 256, 512
NCORES = 8
NTOK = B * T                      # 8192 tokens
TPC = NTOK // NCORES              # 1024 tokens per core
P = 128
HALVES = 2
HTOK = TPC // HALVES              # 512 tokens per half
NB = 8                            # value chunks
CHUNK = C // NB                   # 32768 rows, int16-addressable
NW = 8                            # subgroups per half
WTOK = HTOK // NW                 # 64 tokens per subgroup
NIDX_CALL = NW * P                # 1024 indices per dma_gather call
SPILL_SLABS = 3                   # per half
SPILL_CAP = SPILL_SLABS * P       # 384
MAIN_SLABS = HALVES * NB * NW     # 128
SPILL_TOT = HALVES * SPILL_SLABS  # 6
COLS = NIDX_CALL // 16            # 64 idx16 columns per call
TAILSPLIT = 4                     # last call issued as 4x256-row gathers

_CACHE = {}


def _build_bass():
    import concourse.bass as bass
    import concourse.mybir as mybir
    from concourse import bacc, library_config
    from concourse.tile import TileContext

    fp32 = mybir.dt.float32
    bf16 = mybir.dt.bfloat16
    i16 = mybir.dt.int16
    i32 = mybir.dt.int32
    eq = mybir.AluOpType.is_equal
    mul = mybir.AluOpType.mult
    nc = bacc.Bacc(
        "TRN2", target_bir_lowering=False, debug=False, num_devices=NCORES
    )

    kb = nc.dram_tensor("kb", [C, D], bf16, kind="ExternalInput")
    idxa = nc.dram_tensor("idxa", [P, COLS], i16, kind="ExternalInput")
    idxb = nc.dram_tensor("idxb", [P, (HALVES * NB - 1) * COLS], i16,
                          kind="ExternalInput")
    idxsp = nc.dram_tensor("idxsp", [P, SPILL_TOT], i32, kind="ExternalInput")
    wslot = nc.dram_tensor("wslot", [P, MAIN_SLABS], bf16, kind="ExternalInput")
    tokloc = nc.dram_tensor("tokloc", [P, MAIN_SLABS], i16, kind="ExternalInput")
    wsp = nc.dram_tensor("wsp", [P, SPILL_TOT], bf16, kind="ExternalInput")
    toksp = nc.dram_tensor("toksp", [P, SPILL_TOT], i16, kind="ExternalInput")
    iota64 = nc.dram_tensor("iota64", [P, WTOK], i16, kind="ExternalInput")
    iota512 = nc.dram_tensor("iota512", [P, HTOK], i16, kind="ExternalInput")
    wout = nc.dram_tensor("wout", [P, 2 * E], bf16, kind="ExternalInput")
    out = nc.dram_tensor("out", [TPC, E], bf16, kind="ExternalOutput")

    with TileContext(nc) as tc:
        with (
            tc.tile_pool(name="const", bufs=1) as cpool,
            tc.tile_pool(name="gath", bufs=10) as gpool,
            tc.tile_pool(name="gtail", bufs=1) as gtpool,
            tc.tile_pool(name="mask", bufs=4) as mpool,
            tc.tile_pool(name="spill", bufs=2) as sppool,
            tc.tile_pool(name="spmask", bufs=2) as smpool,
            tc.tile_pool(name="y", bufs=4) as ypool,
            tc.tile_pool(name="o", bufs=4) as opool,
            tc.tile_pool(name="psy", bufs=2, space="PSUM") as psy,
            tc.tile_pool(name="pso", bufs=2, space="PSUM") as pso,
        ):
            # idxa feeds the very first gather: load it first on the HWDGE
            # path (SP) so the first SWDGE desc-gen can start ~1.5us in.
            idxa_sb = cpool.tile([P, COLS], i16)
            nc.sync.dma_start(out=idxa_sb[:], in_=idxa[:, :])
            idxsp_sb = cpool.tile([P, SPILL_TOT], i32)
            nc.sync.dma_start(out=idxsp_sb[:], in_=idxsp[:, :])
            idxb_sb = cpool.tile([P, (HALVES * NB - 1) * COLS], i16)
            nc.sync.dma_start(out=idxb_sb[:], in_=idxb[:, :])
            w_sb = cpool.tile([P, MAIN_SLABS], bf16)
            nc.sync.dma_start(out=w_sb[:], in_=wslot[:, :])
            tl_sb = cpool.tile([P, MAIN_SLABS], i16)
            nc.sync.dma_start(out=tl_sb[:], in_=tokloc[:, :])
            wsp_sb = cpool.tile([P, SPILL_TOT], bf16)
            nc.sync.dma_start(out=wsp_sb[:], in_=wsp[:, :])
            tsp_sb = cpool.tile([P, SPILL_TOT], i16)
            nc.sync.dma_start(out=tsp_sb[:], in_=toksp[:, :])
            io64_sb = cpool.tile([P, WTOK], i16)
            nc.sync.dma_start(out=io64_sb[:], in_=iota64[:, :])
            io512_sb = cpool.tile([P, HTOK], i16)
            nc.sync.dma_start(out=io512_sb[:], in_=iota512[:, :])
            wo_sb = cpool.tile([P, 2 * E], bf16)
            nc.sync.dma_start(out=wo_sb[:], in_=wout[:, :])

            nc.gpsimd.load_library(library_config.mlp)

            def idx_cols(c, lo, hi):
                """idx AP for call c, local column range [lo, hi)."""
                if c == 0:
                    return idxa_sb[:, lo:hi]
                base = (c - 1) * COLS
                return idxb_sb[:, base + lo:base + hi]

            # both halves' spill gathers: one multi-slab indirect DMA each.
            # h0's is issued before the first dma_gather (it fills the DMA
            # engines while call 0's descriptors generate), h1's right after.
            sp_tiles = []
            for hh in range(HALVES):
                sp = sppool.tile([P, SPILL_SLABS, D], bf16, tag=f"sp{hh}")
                for s in range(SPILL_SLABS):
                    col = hh * SPILL_SLABS + s
                    nc.gpsimd.indirect_dma_start(
                        out=sp[:, s, :],
                        out_offset=None,
                        in_=kb[:, :],
                        in_offset=bass.IndirectOffsetOnAxis(
                            ap=idxsp_sb[:, col:col + 1], axis=0
                        ),
                    )
                sp_tiles.append(sp)

            for h in range(HALVES):
                # one psum tile per PAIR of 128-token groups (bank-sized);
                # bucket 0 covers every 64-col window, so its first matmul
                # per bank carries start=True (zeroes the whole 2KB bank)
                # and no memset is needed.  col = g_loc*2P + ch*P + window.
                ytp = []
                for pr in range(HTOK // P // 2):
                    t_ = psy.tile([P, 4 * P], fp32, tag=f"yt{pr}")
                    nc.vector.memset(t_[:], 0.0)
                    ytp.append(t_)

                gs = []
                for b in range(NB):
                    c = h * NB + b
                    if h == 1 and b == NB - 1:
                        # last call: TAILSPLIT small gathers into SEPARATE
                        # tiles (deps are tile-granular) so the epilogue
                        # starts after a 256-row rather than 1024-row wait.
                        sub = NIDX_CALL // TAILSPLIT          # 256 idx
                        subc = COLS // TAILSPLIT              # 16 cols
                        gt = []
                        for q in range(TAILSPLIT):
                            g = gtpool.tile([P, NW // TAILSPLIT, D], bf16,
                                            tag=f"t{q}")
                            nc.gpsimd.dma_gather(
                                out_ap=g[:],
                                in_ap=kb[b * CHUNK:(b + 1) * CHUNK, :],
                                idxs_ap=idx_cols(c, q * subc, (q + 1) * subc),
                                num_idxs=sub,
                                num_idxs_reg=sub,
                                elem_size=D,
                            )
                            gt.append(g)
                        gs.append(gt)
                    else:
                        g = gpool.tile([P, NW, D], bf16, tag="g")
                        nc.gpsimd.dma_gather(
                            out_ap=g[:],
                            in_ap=kb[b * CHUNK:(b + 1) * CHUNK, :],
                            idxs_ap=idx_cols(c, 0, COLS),
                            num_idxs=NIDX_CALL,
                            num_idxs_reg=NIDX_CALL,
                            elem_size=D,
                        )
                        gs.append(g)

                def lhs_slab(b, j, ch):
                    g = gs[b]
                    if isinstance(g, list):
                        per = NW // TAILSPLIT
                        return g[j // per][:, j % per, ch * P:(ch + 1) * P]
                    return g[:, j, ch * P:(ch + 1) * P]

                def do_bucket(b):
                    blk = (h * NB + b) * NW
                    mask = mpool.tile([P, NW, WTOK], bf16, tag="m")
                    nc.vector.tensor_tensor(
                        out=mask[:],
                        in0=io64_sb[:].unsqueeze(1).broadcast_to([P, NW, WTOK]),
                        in1=tl_sb[:, blk:blk + NW].unsqueeze(2)
                            .broadcast_to([P, NW, WTOK]),
                        op=eq,
                    )
                    nc.vector.tensor_tensor(
                        out=mask[:],
                        in0=mask[:],
                        in1=w_sb[:, blk:blk + NW].unsqueeze(2)
                            .broadcast_to([P, NW, WTOK]),
                        op=mul,
                    )
                    for j in range(NW):
                        for ch in range(2):
                            pr = j // 4
                            col = ((j // 2) % 2) * 2 * P + ch * P \
                                + (j % 2) * WTOK
                            nc.tensor.matmul(
                                out=ytp[pr][:, col:col + WTOK],
                                lhsT=lhs_slab(b, j, ch),
                                rhs=mask[:, j, :],
                                start=False,
                                stop=(b == NB - 1 and j % 4 == 3 and ch == 1),
                                skip_group_check=True,
                            )

                def do_spill():
                    msp = smpool.tile([P, SPILL_SLABS, HTOK], bf16, tag="msp")
                    sblk = h * SPILL_SLABS
                    nc.vector.tensor_tensor(
                        out=msp[:],
                        in0=io512_sb[:].unsqueeze(1)
                            .broadcast_to([P, SPILL_SLABS, HTOK]),
                        in1=tsp_sb[:, sblk:sblk + SPILL_SLABS].unsqueeze(2)
                            .broadcast_to([P, SPILL_SLABS, HTOK]),
                        op=eq,
                    )
                    nc.vector.tensor_tensor(
                        out=msp[:],
                        in0=msp[:],
                        in1=wsp_sb[:, sblk:sblk + SPILL_SLABS].unsqueeze(2)
                            .broadcast_to([P, SPILL_SLABS, HTOK]),
                        op=mul,
                    )
                    for s in range(SPILL_SLABS):
                        for ch in range(2):
                            for pr in range(HTOK // P // 2):
                                for g in range(2):
                                    col = g * 2 * P + ch * P
                                    nc.tensor.matmul(
                                        out=ytp[pr][:, col:col + P],
                                        lhsT=sp_tiles[h][:, s,
                                                         ch * P:(ch + 1) * P],
                                        rhs=msp[:, s,
                                                (pr * 2 + g) * P:
                                                (pr * 2 + g + 1) * P],
                                        start=False,
                                        stop=False,
                                        skip_group_check=True,
                                    )

                # bucket 0 first (its matmuls carry the bank-zeroing
                # start=True), then the spill matmuls, then buckets 1..7
                # (bucket 7 carries the stops).
                do_bucket(0)
                do_spill()
                for b in range(1, NB):
                    do_bucket(b)

                # epilogue: yb copies on the Activation engine (DVE builds
                # masks), per 128-token group so stage 2 pipelines tightly.
                ybs = []
                for g4 in range(HTOK // P):
                    yb = ypool.tile([P, 2 * P], bf16, tag=f"yb{g4}")
                    nc.scalar.copy(
                        out=yb[:],
                        in_=ytp[g4 // 2][:, (g4 % 2) * 2 * P:(g4 % 2 + 1) * 2 * P],
                    )
                    ybs.append(yb)
                for g4 in range(HTOK // P):
                    ops = pso.tile([P, E], fp32, tag="ops")
                    for ch in range(2):
                        nc.tensor.matmul(
                            out=ops[:],
                            lhsT=ybs[g4][:, ch * P:(ch + 1) * P],
                            rhs=wo_sb[:, ch * E:(ch + 1) * E],
                            start=(ch == 0),
                            stop=(ch == 1),
                        )
                    osb = opool.tile([P, E], bf16, tag="osb")
                    nc.scalar.copy(out=osb[:], in_=ops[:])
                    row0 = (h * (HTOK // P) + g4) * P
                    nc.sync.dma_start(out=out[row0:row0 + P, :], in_=osb[:])

    nc.compile()
    return nc


def _host_prep(weights, indexes, w_out):
    """Bucket/sort (token,k) pairs per core and build all device-side arrays."""
    import ml_dtypes

    bfloat16 = ml_dtypes.bfloat16
    wflat = np.ascontiguousarray(weights, dtype=np.float32).reshape(NTOK, K)
    iflat = np.ascontiguousarray(indexes).reshape(NTOK, K).astype(np.int64)

    woutT = np.ascontiguousarray(w_out, dtype=np.float32).T      # [D, E]
    wout_host = np.ascontiguousarray(
        woutT.reshape(2, P, E).transpose(1, 0, 2).reshape(P, 2 * E)
    ).astype(bfloat16)
    iota64_h = np.ascontiguousarray(
        np.broadcast_to(np.arange(WTOK, dtype=np.int16), (P, WTOK))
    )
    iota512_h = np.ascontiguousarray(
        np.broadcast_to(np.arange(HTOK, dtype=np.int16), (P, HTOK))
    )

    in_maps = []
    for c in range(NCORES):
        ic = iflat[c * TPC:(c + 1) * TPC].ravel()          # [16384]
        wc = wflat[c * TPC:(c + 1) * TPC].ravel()
        t = np.repeat(np.arange(TPC, dtype=np.int64), K)   # token per pair

        h = t // HTOK
        wsub = (t % HTOK) // WTOK
        b = ic // CHUNK
        key = (h * NB + b) * NW + wsub                     # 0..127 slab id

        order = np.argsort(key, kind="stable")
        ks = key[order]
        iv = ic[order]
        wv = wc[order]
        tv = t[order]
        starts = np.searchsorted(ks, np.arange(MAIN_SLABS))
        rank = np.arange(TPC * K) - starts[ks]

        idx16_host = np.zeros((P, HALVES * NB * COLS), np.int16)
        wslot_host = np.zeros((P, MAIN_SLABS), np.float32)
        tokloc_host = np.zeros((P, MAIN_SLABS), np.int16)
        idxsp_host = np.zeros((P, SPILL_TOT), np.int32)
        wsp_host = np.zeros((P, SPILL_TOT), np.float32)
        toksp_host = np.zeros((P, SPILL_TOT), np.int16)

        main = rank < P
        mk, mr = ks[main], rank[main]
        mi, mw, mt = iv[main], wv[main], tv[main]
        mh = mk // (NB * NW)
        mb = (mk // NW) % NB
        mj = mk % NW
        slot = mj * P + mr                                 # slot within call
        col = (mh * NB + mb) * COLS + slot // 16
        idx_local = (mi - mb * CHUNK).astype(np.int16)
        idx16_host[slot % 16, col] = idx_local             # interp layout
        idx16_host[16 + slot % 16, col] = idx_local        # NEFF Q7 layout
        wslot_host[mr, mk] = mw
        tokloc_host[mr, mk] = (mt - (mh * HTOK + mj * WTOK)).astype(np.int16)

        sh = ks[~main] // (NB * NW)                        # spill half
        si, sw, st = iv[~main], wv[~main], tv[~main]
        for hh in range(HALVES):
            sel = sh == hh
            n = int(sel.sum())
            if n > SPILL_CAP:
                raise ValueError(
                    f"spill overflow: core {c} half {hh} needs {n} > {SPILL_CAP}"
                )
            r = np.arange(n)
            idxsp_host[r % P, hh * SPILL_SLABS + r // P] = si[sel]
            wsp_host[r % P, hh * SPILL_SLABS + r // P] = sw[sel]
            toksp_host[r % P, hh * SPILL_SLABS + r // P] = (
                st[sel] - hh * HTOK
            ).astype(np.int16)

        in_maps.append({
            "idxa": np.ascontiguousarray(idx16_host[:, :COLS]),
            "idxb": np.ascontiguousarray(idx16_host[:, COLS:]),
            "idxsp": idxsp_host,
            "wslot": wslot_host.astype(bfloat16),
            "tokloc": tokloc_host,
            "wsp": wsp_host.astype(bfloat16),
            "toksp": toksp_host,
            "iota64": iota64_h,
            "iota512": iota512_h,
            "wout": wout_host,
        })
    return in_maps


def kernel(weights, indexes, knowledge_base, w_out, b_out):
    import ml_dtypes
    from concourse.bass_utils import run_bass_kernel_spmd

    if "nc" not in _CACHE:
        _CACHE["nc"] = _build_bass()
    nc = _CACHE["nc"]

    kb_host = np.ascontiguousarray(knowledge_base, dtype=np.float32).astype(
        ml_dtypes.bfloat16
    )
    in_maps = _host_prep(weights, indexes, w_out)
    for m in in_maps:
        m["kb"] = kb_host

    res = run_bass_kernel_spmd(nc, in_maps, list(range(NCORES)))
    out = np.concatenate(
        [res.results[c]["out"].astype(np.float32) for c in range(NCORES)],
        axis=0,
    )
    out += np.asarray(b_out, dtype=np.float32)[None, :]
    return out.reshape(B, T, E)


# revision 29
# speedup vs baseline: 1.5120x; 1.0220x over previous
"""Trainium2 Bass kernel for nn_KnowledgeBaseLookup (bucketed dma_gather design).

Computation (see reference):
    lookup = knowledge_base[indexes]            # (B,T,K,D) gather
    y      = einsum('btk,btkd->btd', weights, lookup)
    out    = y @ w_out.T + b_out                # (B,T,E)

Sharding: data-parallel over the B*T token dim across 8 cores; the
knowledge_base table is replicated per core (converted to bf16 on host —
host prep also does the index bucketing/sorting).

Per-core design (1024 tokens, 16384 gathered rows), all-bf16 datapath:
  Rows are gathered with the batched `dma_gather` custom op from a bf16 copy
  of the table, at the price of int16 indices: indices are bucketed by table
  chunk of 32768 rows so chunk-local indices fit in int16, with the chunk
  base carried by the in_ap view.

  Layout: tokens split into 2 halves of 512; each half into 8 subgroups of
  64 tokens.  Pairs are bucketed by (chunk b, half h, subgroup j) into 128
  slabs of up to 128 slots (capacity = the mean occupancy).  Chunks 0-5 are
  fetched with one 2048-index dma_gather each (both halves -> fewer Pool
  desc-gens); chunks 6-7 use per-half 1024-index calls so h0's epilogue
  overlaps h1's last transfers, and the very last call (h1, chunk 7) is
  issued as four 256-row calls so the tail epilogue starts after a short
  transfer.  Overflow pairs go to a per-half spill region of 3 slabs
  gathered by per-slab indirect DMAs (any chunk, int32 indices) whose
  desc-gens are interleaved into the merged calls' Pool slack.

  Reduction: for each slab, a [128,64] bf16 mask M[slot, j] =
  w[slot] * (tokloc[slot] == j) is built on DVE (is_equal on an iota table,
  then multiply; tokloc/weights are host-prepped per slot).  PE matmuls
  lhsT=rows (bf16) x rhs=mask accumulate yT[d, token] into per-(half,pair)
  PSUM banks (zeroed once via memset).  The spill slabs use a 512-wide mask
  over the whole half.  Stage 2 (out_proj) contracts yT (bf16, copied
  per-group PSUM->SBUF on DVE) with w_out.T (bf16) per 128-token group
  (bias is added on host), copies PSUM->SBUF in bf16 on the Activation
  engine, and DMAs out; the host upcasts to fp32.

  The dma_gather Q7 ucode reads index i of a call from the idx tile at
  [16 + i%16, i//16] on the NEFF path (queue 0 channel base), while the
  bass-level interpreter reads [i%16, i//16]; the host writes both bands.
"""

import numpy as np

B, T, K = 4, 2048, 16
C, D, E = 262144, 256, 512
NCORES = 8
NTOK = B * T                      # 8192 tokens
TPC = NTOK // NCORES              # 1024 tokens per core
P = 128
HALVES = 2
HTOK = TPC // HALVES              # 512 tokens per half
NB = 8                            # value chunks
CHUNK = C // NB                   # 32768 rows, int16-addressable
NW = 8                            # subgroups per half
WTOK = HTOK // NW                 # 64 tokens per subgroup
NIDX_CALL = NW * P                # 1024 indices per per-half call
SPILL_SLABS = 3                   # per half
SPILL_CAP = SPILL_SLABS * P       # 384
MAIN_SLABS = HALVES * NB * NW     # 128
SPILL_TOT = HALVES * SPILL_SLABS  # 6
COLS = NIDX_CALL // 16            # 64 idx16 columns per per-half call
NMERGE = 6                        # chunks fetched with merged 2048-idx calls
TAILSPLIT = 4                     # last call issued as 4x256-row gathers
TOTCOLS = HALVES * NB * COLS      # 1024
ACOLS = 2 * COLS                  # merged call 0 = first 128 columns

_CACHE = {}


def _build_bass():
    import concourse.bass as bass
    import concourse.mybir as mybir
    from concourse import bacc, library_config
    from concourse.tile import TileContext

    fp32 = mybir.dt.float32
    bf16 = mybir.dt.bfloat16
    i16 = mybir.dt.int16
    i32 = mybir.dt.int32
    eq = mybir.AluOpType.is_equal
    mul = mybir.AluOpType.mult
    nc = bacc.Bacc(
        "TRN2", target_bir_lowering=False, debug=False, num_devices=NCORES,
        # the merged gather calls emit 2048 descriptors (32KB) in one
        # instruction; the default 16KB dynamic-DMA ring overflows on HW.
        dynamic_dma_scratch_size=49152,
    )

    kb = nc.dram_tensor("kb", [C, D], bf16, kind="ExternalInput")
    idxa = nc.dram_tensor("idxa", [P, ACOLS], i16, kind="ExternalInput")
    idxb = nc.dram_tensor("idxb", [P, TOTCOLS - ACOLS], i16,
                          kind="ExternalInput")
    idxsp = nc.dram_tensor("idxsp", [P, SPILL_TOT], i32, kind="ExternalInput")
    wslot = nc.dram_tensor("wslot", [P, MAIN_SLABS], bf16, kind="ExternalInput")
    tokloc = nc.dram_tensor("tokloc", [P, MAIN_SLABS], i16, kind="ExternalInput")
    wsp = nc.dram_tensor("wsp", [P, SPILL_TOT], bf16, kind="ExternalInput")
    toksp = nc.dram_tensor("toksp", [P, SPILL_TOT], i16, kind="ExternalInput")
    iota64 = nc.dram_tensor("iota64", [P, WTOK], i16, kind="ExternalInput")
    iota512 = nc.dram_tensor("iota512", [P, HTOK], i16, kind="ExternalInput")
    wout = nc.dram_tensor("wout", [P, 2 * E], bf16, kind="ExternalInput")
    out = nc.dram_tensor("out", [TPC, E], bf16, kind="ExternalOutput")

    with TileContext(nc) as tc:
        with (
            tc.tile_pool(name="const", bufs=1) as cpool,
            tc.tile_pool(name="gmerge", bufs=5) as gmpool,
            tc.tile_pool(name="ghalf", bufs=3) as ghpool,
            tc.tile_pool(name="gtail", bufs=1) as gtpool,
            tc.tile_pool(name="mask", bufs=8) as mpool,
            tc.tile_pool(name="spill", bufs=2) as sppool,
            tc.tile_pool(name="spmask", bufs=2) as smpool,
            tc.tile_pool(name="y", bufs=8) as ypool,
            tc.tile_pool(name="o", bufs=8) as opool,
            tc.tile_pool(name="psy", bufs=1, space="PSUM") as psy,
            tc.tile_pool(name="pso", bufs=4, space="PSUM") as pso,
        ):
            # idxa feeds the very first gather: load it first on the HWDGE
            # path (SP) so the first SWDGE desc-gen can start ~1.5us in.
            # load order matters: HWDGE desc-gens serialize at 625ns each and
            # transfers queue behind the first gather, so the tensors that
            # gate desc-gen (idxa/idxsp/idxb) and the mask inputs
            # (io64/tokloc/wslot) go first; the rest can land mid-stream.
            idxa_sb = cpool.tile([P, ACOLS], i16)
            nc.sync.dma_start(out=idxa_sb[:], in_=idxa[:, :])
            idxsp_sb = cpool.tile([P, SPILL_TOT], i32)
            nc.sync.dma_start(out=idxsp_sb[:], in_=idxsp[:, :])
            idxb_sb = cpool.tile([P, TOTCOLS - ACOLS], i16)
            nc.sync.dma_start(out=idxb_sb[:], in_=idxb[:, :])
            io64_sb = cpool.tile([P, WTOK], i16)
            nc.sync.dma_start(out=io64_sb[:], in_=iota64[:, :])
            tl_sb = cpool.tile([P, MAIN_SLABS], i16)
            nc.sync.dma_start(out=tl_sb[:], in_=tokloc[:, :])
            w_sb = cpool.tile([P, MAIN_SLABS], bf16)
            nc.sync.dma_start(out=w_sb[:], in_=wslot[:, :])
            io512_sb = cpool.tile([P, HTOK], i16)
            nc.sync.dma_start(out=io512_sb[:], in_=iota512[:, :])
            wsp_sb = cpool.tile([P, SPILL_TOT], bf16)
            nc.sync.dma_start(out=wsp_sb[:], in_=wsp[:, :])
            tsp_sb = cpool.tile([P, SPILL_TOT], i16)
            nc.sync.dma_start(out=tsp_sb[:], in_=toksp[:, :])
            wo_sb = cpool.tile([P, 2 * E], bf16)
            nc.sync.dma_start(out=wo_sb[:], in_=wout[:, :])

            nc.gpsimd.load_library(library_config.mlp)

            # yT psum banks, one per (half, pair of 128-token groups);
            # col = g_loc*2P + ch*P + window.
            ytp = {}
            for hh in range(HALVES):
                for pr in range(2):
                    t_ = psy.tile([P, 4 * P], fp32, tag=f"yt{hh}{pr}")
                    nc.vector.memset(t_[:], 0.0)
                    ytp[hh, pr] = t_

            def idx_cols(lo, hi):
                """idx AP for global idx16 column range [lo, hi)."""
                if hi <= ACOLS:
                    return idxa_sb[:, lo:hi]
                return idxb_sb[:, lo - ACOLS:hi - ACOLS]

            def spill_dma(hh, s):
                col = hh * SPILL_SLABS + s
                nc.gpsimd.indirect_dma_start(
                    out=sp_tiles[hh][:, s, :],
                    out_offset=None,
                    in_=kb[:, :],
                    in_offset=bass.IndirectOffsetOnAxis(
                        ap=idxsp_sb[:, col:col + 1], axis=0
                    ),
                )

            sp_tiles = []
            for hh in range(HALVES):
                sp = sppool.tile([P, SPILL_SLABS, D], bf16, tag=f"sp{hh}")
                sp_tiles.append(sp)

            # gather calls; spill desc-gens ride the merged calls' Pool slack
            # (a 2048-idx call generates descriptors for 1690ns but transfers
            # for 2912ns, so one 1038ns spill desc-gen fits after each).
            gm = {}
            for b in range(2):
                g = gmpool.tile([P, HALVES * NW, D], bf16, tag="gm")
                for half in range(2):
                    nc.gpsimd.dma_gather(
                        out_ap=g[:, half * NW:(half + 1) * NW, :],
                        in_ap=kb[b * CHUNK:(b + 1) * CHUNK, :],
                        idxs_ap=idx_cols((b * 2 + half) * COLS,
                                         (b * 2 + half + 1) * COLS),
                        num_idxs=NIDX_CALL,
                        num_idxs_reg=NIDX_CALL,
                        elem_size=D,
                    )
                gm[b] = g
            spill_sched = {2: [(0, 0), (0, 1)], 3: [(0, 2)], 4: [(1, 0)],
                           5: [(1, 1), (1, 2)]}
            for b in range(2, NMERGE):
                for hh, s in spill_sched.get(b, []):
                    spill_dma(hh, s)
                g = gmpool.tile([P, HALVES * NW, D], bf16, tag="gm")
                for half in range(2):
                    nc.gpsimd.dma_gather(
                        out_ap=g[:, half * NW:(half + 1) * NW, :],
                        in_ap=kb[b * CHUNK:(b + 1) * CHUNK, :],
                        idxs_ap=idx_cols((b * 2 + half) * COLS,
                                         (b * 2 + half + 1) * COLS),
                        num_idxs=NIDX_CALL,
                        num_idxs_reg=NIDX_CALL,
                        elem_size=D,
                    )
                gm[b] = g

            # per-half calls for chunks 6,7: (h0,b6), (h0,b7), (h1,b6), then
            # (h1,b7) split four ways.  column base 768 + (h*2 + (b-6))*64.
            gh = {}
            for hh, b in ((0, NB - 2), (0, NB - 1), (1, NB - 2)):
                base = NMERGE * 2 * COLS + (hh * 2 + b - (NB - 2)) * COLS
                g = ghpool.tile([P, NW, D], bf16, tag="gh")
                nc.gpsimd.dma_gather(
                    out_ap=g[:],
                    in_ap=kb[b * CHUNK:(b + 1) * CHUNK, :],
                    idxs_ap=idx_cols(base, base + COLS),
                    num_idxs=NIDX_CALL,
                    num_idxs_reg=NIDX_CALL,
                    elem_size=D,
                )
                gh[hh, b] = g
            gt = []
            tbase = NMERGE * 2 * COLS + 3 * COLS
            sub = NIDX_CALL // TAILSPLIT              # 256 idx
            subc = COLS // TAILSPLIT                  # 16 cols
            for q in range(TAILSPLIT):
                g = gtpool.tile([P, NW // TAILSPLIT, D], bf16, tag=f"t{q}")
                nc.gpsimd.dma_gather(
                    out_ap=g[:],
                    in_ap=kb[(NB - 1) * CHUNK:NB * CHUNK, :],
                    idxs_ap=idx_cols(tbase + q * subc, tbase + (q + 1) * subc),
                    num_idxs=sub,
                    num_idxs_reg=sub,
                    elem_size=D,
                )
                gt.append(g)

            def lhs_slab(hh, b, j, ch):
                if b < NMERGE:
                    return gm[b][:, hh * NW + j, ch * P:(ch + 1) * P]
                if hh == 1 and b == NB - 1:
                    per = NW // TAILSPLIT
                    return gt[j // per][:, j % per, ch * P:(ch + 1) * P]
                return gh[hh, b][:, j, ch * P:(ch + 1) * P]

            def build_mask(hh, b, tag, eng=None):
                blk = (b * HALVES + hh) * NW
                mask = mpool.tile([P, NW, WTOK], bf16, tag=tag)
                eng = eng or nc.vector
                eng.tensor_tensor(
                    out=mask[:],
                    in0=io64_sb[:].unsqueeze(1).broadcast_to([P, NW, WTOK]),
                    in1=tl_sb[:, blk:blk + NW].unsqueeze(2)
                        .broadcast_to([P, NW, WTOK]),
                    op=eq,
                )
                eng.tensor_tensor(
                    out=mask[:],
                    in0=mask[:],
                    in1=w_sb[:, blk:blk + NW].unsqueeze(2)
                        .broadcast_to([P, NW, WTOK]),
                    op=mul,
                )
                return mask

            def do_bucket(hh, b, eng=None):
                mask = build_mask(hh, b, "m", eng=eng)
                for j in range(NW):
                    for ch in range(2):
                        pr = j // 4
                        col = ((j // 2) % 2) * 2 * P + ch * P + (j % 2) * WTOK
                        nc.tensor.matmul(
                            out=ytp[hh, pr][:, col:col + WTOK],
                            lhsT=lhs_slab(hh, b, j, ch),
                            rhs=mask[:, j, :],
                            start=False,
                            stop=(b == NB - 1 and j % 4 == 3 and ch == 1),
                            skip_group_check=True,
                        )

            # spill masks depend only on host constants; they are built
            # mid-loop (after buckets 1/2) so DVE's in-order queue reaches
            # them well before the spill matmuls need them, without pushing
            # the per-bucket masks past their data arrival.
            spill_masks = {}

            def build_spill_mask(hh):
                msp = smpool.tile([P, SPILL_SLABS, HTOK], bf16, tag=f"msp{hh}")
                sblk = hh * SPILL_SLABS
                nc.vector.tensor_tensor(
                    out=msp[:],
                    in0=io512_sb[:].unsqueeze(1)
                        .broadcast_to([P, SPILL_SLABS, HTOK]),
                    in1=tsp_sb[:, sblk:sblk + SPILL_SLABS].unsqueeze(2)
                        .broadcast_to([P, SPILL_SLABS, HTOK]),
                    op=eq,
                )
                nc.vector.tensor_tensor(
                    out=msp[:],
                    in0=msp[:],
                    in1=wsp_sb[:, sblk:sblk + SPILL_SLABS].unsqueeze(2)
                        .broadcast_to([P, SPILL_SLABS, HTOK]),
                    op=mul,
                )
                spill_masks[hh] = msp

            def do_spill(hh):
                msp = spill_masks[hh]
                for s in range(SPILL_SLABS):
                    for ch in range(2):
                        for pr in range(2):
                            for g in range(2):
                                col = g * 2 * P + ch * P
                                nc.tensor.matmul(
                                    out=ytp[hh, pr][:, col:col + P],
                                    lhsT=sp_tiles[hh][:, s,
                                                      ch * P:(ch + 1) * P],
                                    rhs=msp[:, s,
                                            (pr * 2 + g) * P:
                                            (pr * 2 + g + 1) * P],
                                    start=False,
                                    stop=False,
                                    skip_group_check=True,
                                )

            for b in range(NMERGE):
                do_bucket(0, b)
                do_bucket(1, b)
                if b == 1:
                    build_spill_mask(0)
                if b == 2:
                    build_spill_mask(1)
                    do_spill(0)
                if b == NMERGE - 1:
                    do_spill(1)
            do_bucket(0, NB - 2)
            do_bucket(0, NB - 1)

            def copy_eng(eng, out_ap, in_ap):
                if eng == "act":
                    nc.scalar.copy(out=out_ap, in_=in_ap)
                else:
                    nc.vector.tensor_copy(out=out_ap, in_=in_ap)

            def epilogue(hh, yb_engines, osb_engines):
                ybs = []
                for g4 in range(HTOK // P):
                    yb = ypool.tile([P, 2 * P], bf16, tag=f"yb{hh}{g4}")
                    src = ytp[hh, g4 // 2][:, (g4 % 2) * 2 * P:
                                           (g4 % 2 + 1) * 2 * P]
                    copy_eng(yb_engines[g4 % len(yb_engines)], yb[:], src)
                    ybs.append(yb)
                for g4 in range(HTOK // P):
                    ops = pso.tile([P, E], fp32, tag="ops")
                    for ch in range(2):
                        nc.tensor.matmul(
                            out=ops[:],
                            lhsT=ybs[g4][:, ch * P:(ch + 1) * P],
                            rhs=wo_sb[:, ch * E:(ch + 1) * E],
                            start=(ch == 0),
                            stop=(ch == 1),
                        )
                    osb = opool.tile([P, E], bf16, tag="osb")
                    copy_eng(osb_engines[g4 % len(osb_engines)],
                             osb[:], ops[:])
                    row0 = (hh * (HTOK // P) + g4) * P
                    nc.sync.dma_start(out=out[row0:row0 + P, :], in_=osb[:])

            # h0's epilogue (on the idle Activation engine) overlaps the
            # (h1,b6)+(h1,b7) transfers; h1's tail masks are built on the
            # Pool engine (idle after desc-gens) so DVE frees up for the
            # tail copies, which split across DVE and Act.
            epilogue(0, ["act"], ["act"])
            do_bucket(1, NB - 2)
            do_bucket(1, NB - 1)
            epilogue(1, ["dve"], ["dve", "act"])

    nc.compile()
    return nc


def _host_prep(weights, indexes, w_out):
    """Bucket/sort (token,k) pairs per core and build all device-side arrays."""
    import ml_dtypes

    bfloat16 = ml_dtypes.bfloat16
    wflat = np.ascontiguousarray(weights, dtype=np.float32).reshape(NTOK, K)
    iflat = np.ascontiguousarray(indexes).reshape(NTOK, K).astype(np.int64)

    woutT = np.ascontiguousarray(w_out, dtype=np.float32).T      # [D, E]
    wout_host = np.ascontiguousarray(
        woutT.reshape(2, P, E).transpose(1, 0, 2).reshape(P, 2 * E)
    ).astype(bfloat16)
    iota64_h = np.ascontiguousarray(
        np.broadcast_to(np.arange(WTOK, dtype=np.int16), (P, WTOK))
    )
    iota512_h = np.ascontiguousarray(
        np.broadcast_to(np.arange(HTOK, dtype=np.int16), (P, HTOK))
    )

    in_maps = []
    for c in range(NCORES):
        ic = iflat[c * TPC:(c + 1) * TPC].ravel()          # [16384]
        wc = wflat[c * TPC:(c + 1) * TPC].ravel()
        t = np.repeat(np.arange(TPC, dtype=np.int64), K)   # token per pair

        h = t // HTOK
        wsub = (t % HTOK) // WTOK
        b = ic // CHUNK
        key = (b * HALVES + h) * NW + wsub                 # 0..127 slab id

        order = np.argsort(key, kind="stable")
        ks = key[order]
        iv = ic[order]
        wv = wc[order]
        tv = t[order]
        starts = np.searchsorted(ks, np.arange(MAIN_SLABS))
        rank = np.arange(TPC * K) - starts[ks]

        idx16_host = np.zeros((P, TOTCOLS), np.int16)
        wslot_host = np.zeros((P, MAIN_SLABS), np.float32)
        tokloc_host = np.zeros((P, MAIN_SLABS), np.int16)
        idxsp_host = np.zeros((P, SPILL_TOT), np.int32)
        wsp_host = np.zeros((P, SPILL_TOT), np.float32)
        toksp_host = np.zeros((P, SPILL_TOT), np.int16)

        main = rank < P
        mk, mr = ks[main], rank[main]
        mi, mw, mt = iv[main], wv[main], tv[main]
        mb = mk // (HALVES * NW)
        mh = (mk // NW) % HALVES
        mj = mk % NW
        # slot within the call, and the call's global idx16 column base
        merged = mb < NMERGE
        slot = np.where(merged, (mh * NW + mj) * P + mr, mj * P + mr)
        cbase = np.where(
            merged,
            mb * 2 * COLS,
            NMERGE * 2 * COLS + (mh * 2 + (mb - (NB - 2))) * COLS,
        )
        col = cbase + slot // 16
        idx_local = (mi - mb * CHUNK).astype(np.int16)
        idx16_host[slot % 16, col] = idx_local             # interp layout
        idx16_host[16 + slot % 16, col] = idx_local        # NEFF Q7 layout
        wslot_host[mr, mk] = mw
        tokloc_host[mr, mk] = (mt - (mh * HTOK + mj * WTOK)).astype(np.int16)

        sh = (ks[~main] // NW) % HALVES                    # spill half
        si, sw, st = iv[~main], wv[~main], tv[~main]
        for hh in range(HALVES):
            sel = sh == hh
            n = int(sel.sum())
            if n > SPILL_CAP:
                raise ValueError(
                    f"spill overflow: core {c} half {hh} needs {n} > {SPILL_CAP}"
                )
            r = np.arange(n)
            idxsp_host[r % P, hh * SPILL_SLABS + r // P] = si[sel]
            wsp_host[r % P, hh * SPILL_SLABS + r // P] = sw[sel]
            toksp_host[r % P, hh * SPILL_SLABS + r // P] = (
                st[sel] - hh * HTOK
            ).astype(np.int16)

        in_maps.append({
            "idxa": np.ascontiguousarray(idx16_host[:, :ACOLS]),
            "idxb": np.ascontiguousarray(idx16_host[:, ACOLS:]),
            "idxsp": idxsp_host,
            "wslot": wslot_host.astype(bfloat16),
            "tokloc": tokloc_host,
            "wsp": wsp_host.astype(bfloat16),
            "toksp": toksp_host,
            "iota64": iota64_h,
            "iota512": iota512_h,
            "wout": wout_host,
        })
    return in_maps


def kernel(weights, indexes, knowledge_base, w_out, b_out):
    import ml_dtypes
    from concourse.bass_utils import run_bass_kernel_spmd

    if "nc" not in _CACHE:
        _CACHE["nc"] = _build_bass()
    nc = _CACHE["nc"]

    kb_host = np.ascontiguousarray(knowledge_base, dtype=np.float32).astype(
        ml_dtypes.bfloat16
    )
    in_maps = _host_prep(weights, indexes, w_out)
    for m in in_maps:
        m["kb"] = kb_host

    res = run_bass_kernel_spmd(nc, in_maps, list(range(NCORES)))
    out = np.concatenate(
        [res.results[c]["out"].astype(np.float32) for c in range(NCORES)],
        axis=0,
    )
    out += np.asarray(b_out, dtype=np.float32)[None, :]
    return out.reshape(B, T, E)


# revision 31
# speedup vs baseline: 1.5622x; 1.0332x over previous
"""Trainium2 Bass kernel for nn_KnowledgeBaseLookup (bucketed dma_gather design).

Computation (see reference):
    lookup = knowledge_base[indexes]            # (B,T,K,D) gather
    y      = einsum('btk,btkd->btd', weights, lookup)
    out    = y @ w_out.T + b_out                # (B,T,E)

Sharding: data-parallel over the B*T token dim across 8 cores; the
knowledge_base table is replicated per core (converted to bf16 on host —
host prep also does the index bucketing/sorting).

Per-core design (1024 tokens, 16384 gathered rows), all-bf16 datapath:
  Rows are gathered with the batched `dma_gather` custom op from a bf16 copy
  of the table, at the price of int16 indices: indices are bucketed by table
  chunk of 32768 rows so chunk-local indices fit in int16, with the chunk
  base carried by the in_ap view.

  Layout: tokens split into 2 halves of 512; each half into 8 subgroups of
  64 tokens.  Pairs are bucketed by (chunk b, half h, subgroup j) into 128
  slabs of up to 128 slots (capacity = the mean occupancy).  Chunks 0-5 are
  fetched with one 2048-index dma_gather each (both halves -> fewer Pool
  desc-gens); chunks 6-7 use per-half 1024-index calls so h0's epilogue
  overlaps h1's last transfers, and the very last call (h1, chunk 7) is
  issued as four 256-row calls so the tail epilogue starts after a short
  transfer.  Overflow pairs go to a per-half spill region of 3 slabs
  gathered by per-slab indirect DMAs (any chunk, int32 indices) whose
  desc-gens are interleaved into the merged calls' Pool slack.

  Reduction: for each slab, a [128,64] bf16 mask M[slot, j] =
  w[slot] * (tokloc[slot] == j) is built on DVE (is_equal on an iota table,
  then multiply; tokloc/weights are host-prepped per slot).  PE matmuls
  lhsT=rows (bf16) x rhs=mask accumulate yT[d, token] into per-(half,pair)
  PSUM banks (zeroed once via memset).  The spill slabs use a 512-wide mask
  over the whole half.  Stage 2 (out_proj) contracts yT (bf16, copied
  per-group PSUM->SBUF on DVE) with w_out.T (bf16) per 128-token group
  (bias is added on host), copies PSUM->SBUF in bf16 on the Activation
  engine, and DMAs out; the host upcasts to fp32.

  The dma_gather Q7 ucode reads index i of a call from the idx tile at
  [16 + i%16, i//16] on the NEFF path (queue 0 channel base), while the
  bass-level interpreter reads [i%16, i//16]; the host writes both bands.
"""

import numpy as np

B, T, K = 4, 2048, 16
C, D, E = 262144, 256, 512
NCORES = 8
NTOK = B * T                      # 8192 tokens
TPC = NTOK // NCORES              # 1024 tokens per core
P = 128
HALVES = 2
HTOK = TPC // HALVES              # 512 tokens per half
NB = 8                            # value chunks
CHUNK = C // NB                   # 32768 rows, int16-addressable
NW = 8                            # subgroups per half
WTOK = HTOK // NW                 # 64 tokens per subgroup
NIDX_CALL = NW * P                # 1024 indices per per-half call
SPILL_SLABS = 3                   # per half
SPILL_CAP = SPILL_SLABS * P       # 384
MAIN_SLABS = HALVES * NB * NW     # 128
SPILL_TOT = HALVES * SPILL_SLABS  # 6
COLS = NIDX_CALL // 16            # 64 idx16 columns per per-half call
NMERGE = 6                        # chunks fetched with merged 2048-idx calls
TAILSPLIT = 2                     # last call issued as 2x512-row gathers
TOTCOLS = HALVES * NB * COLS      # 1024
ACOLS = 2 * COLS                  # merged call 0 = first 128 columns

_CACHE = {}


def _build_bass():
    import concourse.bass as bass
    import concourse.mybir as mybir
    from concourse import bacc, library_config
    from concourse.tile import TileContext

    fp32 = mybir.dt.float32
    bf16 = mybir.dt.bfloat16
    i16 = mybir.dt.int16
    i32 = mybir.dt.int32
    eq = mybir.AluOpType.is_equal
    mul = mybir.AluOpType.mult
    nc = bacc.Bacc(
        "TRN2", target_bir_lowering=False, debug=False, num_devices=NCORES,
        # the merged gather calls emit 2048 descriptors (32KB) in one
        # instruction; the default 16KB dynamic-DMA ring overflows on HW.
        dynamic_dma_scratch_size=49152,
    )

    kb = nc.dram_tensor("kb", [C, D], bf16, kind="ExternalInput")
    idxa = nc.dram_tensor("idxa", [P, ACOLS], i16, kind="ExternalInput")
    idxb = nc.dram_tensor("idxb", [P, TOTCOLS - ACOLS], i16,
                          kind="ExternalInput")
    idxsp = nc.dram_tensor("idxsp", [P, SPILL_TOT], i32, kind="ExternalInput")
    wslot = nc.dram_tensor("wslot", [P, MAIN_SLABS], bf16, kind="ExternalInput")
    tokloc = nc.dram_tensor("tokloc", [P, MAIN_SLABS], i16, kind="ExternalInput")
    wsp = nc.dram_tensor("wsp", [P, SPILL_TOT], bf16, kind="ExternalInput")
    toksp = nc.dram_tensor("toksp", [P, SPILL_TOT], i16, kind="ExternalInput")
    iota64 = nc.dram_tensor("iota64", [P, WTOK], i16, kind="ExternalInput")
    iota512 = nc.dram_tensor("iota512", [P, HTOK], i16, kind="ExternalInput")
    wout = nc.dram_tensor("wout", [P, 2 * E], bf16, kind="ExternalInput")
    out = nc.dram_tensor("out", [TPC, E], bf16, kind="ExternalOutput")

    with TileContext(nc) as tc:
        with (
            tc.tile_pool(name="const", bufs=1) as cpool,
            tc.tile_pool(name="gmerge", bufs=5) as gmpool,
            tc.tile_pool(name="ghalf", bufs=3) as ghpool,
            tc.tile_pool(name="gtail", bufs=1) as gtpool,
            tc.tile_pool(name="mask", bufs=8) as mpool,
            tc.tile_pool(name="spill", bufs=2) as sppool,
            tc.tile_pool(name="spmask", bufs=2) as smpool,
            tc.tile_pool(name="y", bufs=8) as ypool,
            tc.tile_pool(name="o", bufs=1) as opool,
            tc.tile_pool(name="psy", bufs=1, space="PSUM") as psy,
            tc.tile_pool(name="pso", bufs=4, space="PSUM") as pso,
        ):
            # idxa feeds the very first gather: load it first on the HWDGE
            # path (SP) so the first SWDGE desc-gen can start ~1.5us in.
            # load order matters: HWDGE desc-gens serialize at 625ns each and
            # transfers queue behind the first gather, so the tensors that
            # gate desc-gen (idxa/idxsp/idxb) and the mask inputs
            # (io64/tokloc/wslot) go first; the rest can land mid-stream.
            idxa_sb = cpool.tile([P, ACOLS], i16)
            nc.sync.dma_start(out=idxa_sb[:], in_=idxa[:, :])
            idxsp_sb = cpool.tile([P, SPILL_TOT], i32)
            nc.sync.dma_start(out=idxsp_sb[:], in_=idxsp[:, :])
            idxb_sb = cpool.tile([P, TOTCOLS - ACOLS], i16)
            nc.sync.dma_start(out=idxb_sb[:], in_=idxb[:, :])
            io64_sb = cpool.tile([P, WTOK], i16)
            nc.sync.dma_start(out=io64_sb[:], in_=iota64[:, :])
            tl_sb = cpool.tile([P, MAIN_SLABS], i16)
            nc.sync.dma_start(out=tl_sb[:], in_=tokloc[:, :])
            w_sb = cpool.tile([P, MAIN_SLABS], bf16)
            nc.sync.dma_start(out=w_sb[:], in_=wslot[:, :])
            io512_sb = cpool.tile([P, HTOK], i16)
            nc.sync.dma_start(out=io512_sb[:], in_=iota512[:, :])
            wsp_sb = cpool.tile([P, SPILL_TOT], bf16)
            nc.sync.dma_start(out=wsp_sb[:], in_=wsp[:, :])
            tsp_sb = cpool.tile([P, SPILL_TOT], i16)
            nc.sync.dma_start(out=tsp_sb[:], in_=toksp[:, :])
            wo_sb = cpool.tile([P, 2 * E], bf16)
            nc.sync.dma_start(out=wo_sb[:], in_=wout[:, :])

            nc.gpsimd.load_library(library_config.mlp)

            # yT psum banks, one per (half, pair of 128-token groups);
            # col = g_loc*2P + ch*P + window.
            ytp = {}
            for hh in range(HALVES):
                for pr in range(2):
                    t_ = psy.tile([P, 4 * P], fp32, tag=f"yt{hh}{pr}")
                    nc.vector.memset(t_[:], 0.0)
                    ytp[hh, pr] = t_

            def idx_cols(lo, hi):
                """idx AP for global idx16 column range [lo, hi)."""
                if hi <= ACOLS:
                    return idxa_sb[:, lo:hi]
                return idxb_sb[:, lo - ACOLS:hi - ACOLS]

            def spill_dma(hh, s):
                col = hh * SPILL_SLABS + s
                nc.gpsimd.indirect_dma_start(
                    out=sp_tiles[hh][:, s, :],
                    out_offset=None,
                    in_=kb[:, :],
                    in_offset=bass.IndirectOffsetOnAxis(
                        ap=idxsp_sb[:, col:col + 1], axis=0
                    ),
                )

            sp_tiles = []
            for hh in range(HALVES):
                sp = sppool.tile([P, SPILL_SLABS, D], bf16, tag=f"sp{hh}")
                sp_tiles.append(sp)

            # gather calls; spill desc-gens ride the merged calls' Pool slack
            # (a 2048-idx call generates descriptors for 1690ns but transfers
            # for 2912ns, so one 1038ns spill desc-gen fits after each).
            gm = {}
            for b in range(2):
                g = gmpool.tile([P, HALVES * NW, D], bf16, tag="gm")
                for half in range(2):
                    nc.gpsimd.dma_gather(
                        out_ap=g[:, half * NW:(half + 1) * NW, :],
                        in_ap=kb[b * CHUNK:(b + 1) * CHUNK, :],
                        idxs_ap=idx_cols((b * 2 + half) * COLS,
                                         (b * 2 + half + 1) * COLS),
                        num_idxs=NIDX_CALL,
                        num_idxs_reg=NIDX_CALL,
                        elem_size=D,
                    )
                gm[b] = g
            spill_sched = {2: [(0, 0), (0, 1)], 3: [(0, 2)], 4: [(1, 0)],
                           5: [(1, 1), (1, 2)]}
            for b in range(2, NMERGE):
                for hh, s in spill_sched.get(b, []):
                    spill_dma(hh, s)
                g = gmpool.tile([P, HALVES * NW, D], bf16, tag="gm")
                for half in range(2):
                    nc.gpsimd.dma_gather(
                        out_ap=g[:, half * NW:(half + 1) * NW, :],
                        in_ap=kb[b * CHUNK:(b + 1) * CHUNK, :],
                        idxs_ap=idx_cols((b * 2 + half) * COLS,
                                         (b * 2 + half + 1) * COLS),
                        num_idxs=NIDX_CALL,
                        num_idxs_reg=NIDX_CALL,
                        elem_size=D,
                    )
                gm[b] = g

            # per-half calls for chunks 6,7: (h0,b6), (h0,b7), (h1,b6), then
            # (h1,b7) split four ways.  column base 768 + (h*2 + (b-6))*64.
            gh = {}
            for hh, b in ((0, NB - 2), (0, NB - 1), (1, NB - 2)):
                base = NMERGE * 2 * COLS + (hh * 2 + b - (NB - 2)) * COLS
                g = ghpool.tile([P, NW, D], bf16, tag="gh")
                nc.gpsimd.dma_gather(
                    out_ap=g[:],
                    in_ap=kb[b * CHUNK:(b + 1) * CHUNK, :],
                    idxs_ap=idx_cols(base, base + COLS),
                    num_idxs=NIDX_CALL,
                    num_idxs_reg=NIDX_CALL,
                    elem_size=D,
                )
                gh[hh, b] = g
            gt = []
            tbase = NMERGE * 2 * COLS + 3 * COLS
            sub = NIDX_CALL // TAILSPLIT              # 256 idx
            subc = COLS // TAILSPLIT                  # 16 cols
            for q in range(TAILSPLIT):
                g = gtpool.tile([P, NW // TAILSPLIT, D], bf16, tag=f"t{q}")
                nc.gpsimd.dma_gather(
                    out_ap=g[:],
                    in_ap=kb[(NB - 1) * CHUNK:NB * CHUNK, :],
                    idxs_ap=idx_cols(tbase + q * subc, tbase + (q + 1) * subc),
                    num_idxs=sub,
                    num_idxs_reg=sub,
                    elem_size=D,
                )
                gt.append(g)

            def lhs_slab(hh, b, j, ch):
                if b < NMERGE:
                    return gm[b][:, hh * NW + j, ch * P:(ch + 1) * P]
                if hh == 1 and b == NB - 1:
                    per = NW // TAILSPLIT
                    return gt[j // per][:, j % per, ch * P:(ch + 1) * P]
                return gh[hh, b][:, j, ch * P:(ch + 1) * P]

            def build_mask(hh, b, tag, eng=None):
                blk = (b * HALVES + hh) * NW
                mask = mpool.tile([P, NW, WTOK], bf16, tag=tag)
                eng = eng or nc.vector
                eng.tensor_tensor(
                    out=mask[:],
                    in0=io64_sb[:].unsqueeze(1).broadcast_to([P, NW, WTOK]),
                    in1=tl_sb[:, blk:blk + NW].unsqueeze(2)
                        .broadcast_to([P, NW, WTOK]),
                    op=eq,
                )
                eng.tensor_tensor(
                    out=mask[:],
                    in0=mask[:],
                    in1=w_sb[:, blk:blk + NW].unsqueeze(2)
                        .broadcast_to([P, NW, WTOK]),
                    op=mul,
                )
                return mask

            def do_bucket(hh, b, eng=None):
                mask = build_mask(hh, b, "m", eng=eng)
                for j in range(NW):
                    for ch in range(2):
                        pr = j // 4
                        col = ((j // 2) % 2) * 2 * P + ch * P + (j % 2) * WTOK
                        nc.tensor.matmul(
                            out=ytp[hh, pr][:, col:col + WTOK],
                            lhsT=lhs_slab(hh, b, j, ch),
                            rhs=mask[:, j, :],
                            start=False,
                            stop=(b == NB - 1 and j % 4 == 3 and ch == 1),
                            skip_group_check=True,
                        )

            # spill masks depend only on host constants; they are built
            # mid-loop (after buckets 1/2) so DVE's in-order queue reaches
            # them well before the spill matmuls need them, without pushing
            # the per-bucket masks past their data arrival.
            spill_masks = {}

            def build_spill_mask(hh):
                msp = smpool.tile([P, SPILL_SLABS, HTOK], bf16, tag=f"msp{hh}")
                sblk = hh * SPILL_SLABS
                nc.vector.tensor_tensor(
                    out=msp[:],
                    in0=io512_sb[:].unsqueeze(1)
                        .broadcast_to([P, SPILL_SLABS, HTOK]),
                    in1=tsp_sb[:, sblk:sblk + SPILL_SLABS].unsqueeze(2)
                        .broadcast_to([P, SPILL_SLABS, HTOK]),
                    op=eq,
                )
                nc.vector.tensor_tensor(
                    out=msp[:],
                    in0=msp[:],
                    in1=wsp_sb[:, sblk:sblk + SPILL_SLABS].unsqueeze(2)
                        .broadcast_to([P, SPILL_SLABS, HTOK]),
                    op=mul,
                )
                spill_masks[hh] = msp

            def do_spill(hh):
                msp = spill_masks[hh]
                for s in range(SPILL_SLABS):
                    for ch in range(2):
                        for pr in range(2):
                            for g in range(2):
                                col = g * 2 * P + ch * P
                                nc.tensor.matmul(
                                    out=ytp[hh, pr][:, col:col + P],
                                    lhsT=sp_tiles[hh][:, s,
                                                      ch * P:(ch + 1) * P],
                                    rhs=msp[:, s,
                                            (pr * 2 + g) * P:
                                            (pr * 2 + g + 1) * P],
                                    start=False,
                                    stop=False,
                                    skip_group_check=True,
                                )

            for b in range(NMERGE):
                do_bucket(0, b)
                do_bucket(1, b)
                if b == 1:
                    build_spill_mask(0)
                if b == 2:
                    build_spill_mask(1)
                    do_spill(0)
                if b == NMERGE - 1:
                    do_spill(1)
            do_bucket(0, NB - 2)
            do_bucket(0, NB - 1)

            def copy_eng(eng, out_ap, in_ap):
                if eng == "act":
                    nc.scalar.copy(out=out_ap, in_=in_ap)
                else:
                    nc.vector.tensor_copy(out=out_ap, in_=in_ap)

            def epilogue(hh, yb_engines, osb_engines):
                ybs = []
                for g4 in range(HTOK // P):
                    yb = ypool.tile([P, 2 * P], bf16, tag=f"yb{hh}{g4}")
                    src = ytp[hh, g4 // 2][:, (g4 % 2) * 2 * P:
                                           (g4 % 2 + 1) * 2 * P]
                    copy_eng(yb_engines[g4 % len(yb_engines)], yb[:], src)
                    ybs.append(yb)
                # stores are paired (two 128-token groups per dma_start):
                # HWDGE desc-gens cost 625ns each, run after the store's sem
                # wait, and serialize -- fewer, bigger stores shorten the tail
                for pair in range(2):
                    ot = opool.tile([P, 2, E], bf16, tag=f"os{hh}{pair}")
                    for gi in range(2):
                        g4 = pair * 2 + gi
                        ops = pso.tile([P, E], fp32, tag="ops")
                        for ch in range(2):
                            nc.tensor.matmul(
                                out=ops[:],
                                lhsT=ybs[g4][:, ch * P:(ch + 1) * P],
                                rhs=wo_sb[:, ch * E:(ch + 1) * E],
                                start=(ch == 0),
                                stop=(ch == 1),
                            )
                        copy_eng(osb_engines[g4 % len(osb_engines)],
                                 ot[:, gi, :], ops[:])
                    row0 = (hh * 4 + pair * 2) * P
                    nc.sync.dma_start(
                        out=out[row0:row0 + 2 * P, :]
                            .rearrange("(c p) e -> p c e", c=2),
                        in_=ot[:],
                    )

            # h0's epilogue (on the idle Activation engine) overlaps the
            # (h1,b6)+(h1,b7) transfers; h1's tail masks are built on the
            # Pool engine (idle after desc-gens) so DVE frees up for the
            # tail copies, which split across DVE and Act.
            epilogue(0, ["act"], ["act"])
            do_bucket(1, NB - 2)
            do_bucket(1, NB - 1)
            epilogue(1, ["dve"], ["dve", "act"])

    nc.compile()
    return nc


def _host_prep(weights, indexes, w_out):
    """Bucket/sort (token,k) pairs per core and build all device-side arrays."""
    import ml_dtypes

    bfloat16 = ml_dtypes.bfloat16
    wflat = np.ascontiguousarray(weights, dtype=np.float32).reshape(NTOK, K)
    iflat = np.ascontiguousarray(indexes).reshape(NTOK, K).astype(np.int64)

    woutT = np.ascontiguousarray(w_out, dtype=np.float32).T      # [D, E]
    wout_host = np.ascontiguousarray(
        woutT.reshape(2, P, E).transpose(1, 0, 2).reshape(P, 2 * E)
    ).astype(bfloat16)
    iota64_h = np.ascontiguousarray(
        np.broadcast_to(np.arange(WTOK, dtype=np.int16), (P, WTOK))
    )
    iota512_h = np.ascontiguousarray(
        np.broadcast_to(np.arange(HTOK, dtype=np.int16), (P, HTOK))
    )

    in_maps = []
    for c in range(NCORES):
        ic = iflat[c * TPC:(c + 1) * TPC].ravel()          # [16384]
        wc = wflat[c * TPC:(c + 1) * TPC].ravel()
        t = np.repeat(np.arange(TPC, dtype=np.int64), K)   # token per pair

        h = t // HTOK
        wsub = (t % HTOK) // WTOK
        b = ic // CHUNK
        key = (b * HALVES + h) * NW + wsub                 # 0..127 slab id

        order = np.argsort(key, kind="stable")
        ks = key[order]
        iv = ic[order]
        wv = wc[order]
        tv = t[order]
        starts = np.searchsorted(ks, np.arange(MAIN_SLABS))
        rank = np.arange(TPC * K) - starts[ks]

        idx16_host = np.zeros((P, TOTCOLS), np.int16)
        wslot_host = np.zeros((P, MAIN_SLABS), np.float32)
        tokloc_host = np.zeros((P, MAIN_SLABS), np.int16)
        idxsp_host = np.zeros((P, SPILL_TOT), np.int32)
        wsp_host = np.zeros((P, SPILL_TOT), np.float32)
        toksp_host = np.zeros((P, SPILL_TOT), np.int16)

        main = rank < P
        mk, mr = ks[main], rank[main]
        mi, mw, mt = iv[main], wv[main], tv[main]
        mb = mk // (HALVES * NW)
        mh = (mk // NW) % HALVES
        mj = mk % NW
        # slot within the call, and the call's global idx16 column base
        merged = mb < NMERGE
        slot = np.where(merged, (mh * NW + mj) * P + mr, mj * P + mr)
        cbase = np.where(
            merged,
            mb * 2 * COLS,
            NMERGE * 2 * COLS + (mh * 2 + (mb - (NB - 2))) * COLS,
        )
        col = cbase + slot // 16
        idx_local = (mi - mb * CHUNK).astype(np.int16)
        idx16_host[slot % 16, col] = idx_local             # interp layout
        idx16_host[16 + slot % 16, col] = idx_local        # NEFF Q7 layout
        wslot_host[mr, mk] = mw
        tokloc_host[mr, mk] = (mt - (mh * HTOK + mj * WTOK)).astype(np.int16)

        sh = (ks[~main] // NW) % HALVES                    # spill half
        si, sw, st = iv[~main], wv[~main], tv[~main]
        for hh in range(HALVES):
            sel = sh == hh
            n = int(sel.sum())
            if n > SPILL_CAP:
                raise ValueError(
                    f"spill overflow: core {c} half {hh} needs {n} > {SPILL_CAP}"
                )
            r = np.arange(n)
            idxsp_host[r % P, hh * SPILL_SLABS + r // P] = si[sel]
            wsp_host[r % P, hh * SPILL_SLABS + r // P] = sw[sel]
            toksp_host[r % P, hh * SPILL_SLABS + r // P] = (
                st[sel] - hh * HTOK
            ).astype(np.int16)

        in_maps.append({
            "idxa": np.ascontiguousarray(idx16_host[:, :ACOLS]),
            "idxb": np.ascontiguousarray(idx16_host[:, ACOLS:]),
            "idxsp": idxsp_host,
            "wslot": wslot_host.astype(bfloat16),
            "tokloc": tokloc_host,
            "wsp": wsp_host.astype(bfloat16),
            "toksp": toksp_host,
            "iota64": iota64_h,
            "iota512": iota512_h,
            "wout": wout_host,
        })
    return in_maps


def kernel(weights, indexes, knowledge_base, w_out, b_out):
    import ml_dtypes
    from concourse.bass_utils import run_bass_kernel_spmd

    if "nc" not in _CACHE:
        _CACHE["nc"] = _build_bass()
    nc = _CACHE["nc"]

    kb_host = np.ascontiguousarray(knowledge_base, dtype=np.float32).astype(
        ml_dtypes.bfloat16
    )
    in_maps = _host_prep(weights, indexes, w_out)
    for m in in_maps:
        m["kb"] = kb_host

    res = run_bass_kernel_spmd(nc, in_maps, list(range(NCORES)))
    out = np.concatenate(
        [res.results[c]["out"].astype(np.float32) for c in range(NCORES)],
        axis=0,
    )
    out += np.asarray(b_out, dtype=np.float32)[None, :]
    return out.reshape(B, T, E)


# revision 40
# speedup vs baseline: 1.5752x; 1.0083x over previous
"""Trainium2 Bass kernel for nn_KnowledgeBaseLookup (bucketed dma_gather design).

Computation (see reference):
    lookup = knowledge_base[indexes]            # (B,T,K,D) gather
    y      = einsum('btk,btkd->btd', weights, lookup)
    out    = y @ w_out.T + b_out                # (B,T,E)

Sharding: data-parallel over the B*T token dim across 8 cores; the
knowledge_base table is replicated per core (converted to bf16 on host —
host prep also does the index bucketing/sorting).

Per-core design (1024 tokens, 16384 gathered rows), all-bf16 datapath:
  Rows are gathered with the batched `dma_gather` custom op from a bf16 copy
  of the table, at the price of int16 indices: indices are bucketed by table
  chunk of 32768 rows so chunk-local indices fit in int16, with the chunk
  base carried by the in_ap view.

  Layout: tokens split into 2 halves of 512; each half into 8 subgroups of
  64 tokens.  Pairs are bucketed by (chunk b, half h, subgroup j) into 128
  slabs of up to 128 slots (capacity = the mean occupancy).  Chunks 0-5 are
  fetched with one 2048-index dma_gather each (both halves -> fewer Pool
  desc-gens); chunks 6-7 use per-half 1024-index calls so h0's epilogue
  overlaps h1's last transfers, and the very last call (h1, chunk 7) is
  issued as four 256-row calls so the tail epilogue starts after a short
  transfer.  Overflow pairs go to a per-half spill region of 3 slabs
  gathered by per-slab indirect DMAs (any chunk, int32 indices) whose
  desc-gens are interleaved into the merged calls' Pool slack.

  Reduction: for each slab, a [128,64] bf16 mask M[slot, j] =
  w[slot] * (tokloc[slot] == j) is built on DVE (is_equal on an iota table,
  then multiply; tokloc/weights are host-prepped per slot).  PE matmuls
  lhsT=rows (bf16) x rhs=mask accumulate yT[d, token] into per-(half,pair)
  PSUM banks (zeroed once via memset).  The spill slabs use a 512-wide mask
  over the whole half.  Stage 2 (out_proj) contracts yT (bf16, copied
  per-group PSUM->SBUF on DVE) with w_out.T (bf16) per 128-token group
  (bias is added on host), copies PSUM->SBUF in bf16 on the Activation
  engine, and DMAs out; the host upcasts to fp32.

  The dma_gather Q7 ucode reads index i of a call from the idx tile at
  [16 + i%16, i//16] on the NEFF path (queue 0 channel base), while the
  bass-level interpreter reads [i%16, i//16]; the host writes both bands.
"""

import numpy as np

B, T, K = 4, 2048, 16
C, D, E = 262144, 256, 512
NCORES = 8
NTOK = B * T                      # 8192 tokens
TPC = NTOK // NCORES              # 1024 tokens per core
P = 128
HALVES = 2
HTOK = TPC // HALVES              # 512 tokens per half
NB = 8                            # value chunks
CHUNK = C // NB                   # 32768 rows, int16-addressable
NW = 8                            # subgroups per half
WTOK = HTOK // NW                 # 64 tokens per subgroup
NIDX_CALL = NW * P                # 1024 indices per per-half call
SPILL_SLABS = 3                   # per half
SPILL_CAP = SPILL_SLABS * P       # 384
MAIN_SLABS = HALVES * NB * NW     # 128
SPILL_TOT = HALVES * SPILL_SLABS  # 6
COLS = NIDX_CALL // 16            # 64 idx16 columns per per-half call
TAILSPLIT = 2                     # last call issued as 2x512-row gathers
TOTCOLS = HALVES * NB * COLS      # 1024
ACOLS = 2 * COLS                  # calls 0-1 = first 128 columns

_CACHE = {}


def _build_bass():
    import concourse.bass as bass
    import concourse.mybir as mybir
    from concourse import bacc, library_config
    from concourse.tile import TileContext

    fp32 = mybir.dt.float32
    bf16 = mybir.dt.bfloat16
    i16 = mybir.dt.int16
    i32 = mybir.dt.int32
    eq = mybir.AluOpType.is_equal
    mul = mybir.AluOpType.mult
    nc = bacc.Bacc(
        "TRN2", target_bir_lowering=False, debug=False, num_devices=NCORES,
        # the merged gather calls emit 2048 descriptors (32KB) in one
        # instruction; the default 16KB dynamic-DMA ring overflows on HW.
        dynamic_dma_scratch_size=49152,
    )

    kb = nc.dram_tensor("kb", [C, D], bf16, kind="ExternalInput")
    idxa = nc.dram_tensor("idxa", [P, ACOLS], i16, kind="ExternalInput")
    idxb = nc.dram_tensor("idxb", [P, TOTCOLS - ACOLS], i16,
                          kind="ExternalInput")
    idxsp = nc.dram_tensor("idxsp", [P, SPILL_TOT], i32, kind="ExternalInput")
    wslot = nc.dram_tensor("wslot", [P, MAIN_SLABS], bf16, kind="ExternalInput")
    tokloc = nc.dram_tensor("tokloc", [P, MAIN_SLABS], i16, kind="ExternalInput")
    wsp = nc.dram_tensor("wsp", [P, SPILL_TOT], bf16, kind="ExternalInput")
    toksp = nc.dram_tensor("toksp", [P, SPILL_TOT], i16, kind="ExternalInput")
    iota64 = nc.dram_tensor("iota64", [P, WTOK], i16, kind="ExternalInput")
    iota512 = nc.dram_tensor("iota512", [P, HTOK], i16, kind="ExternalInput")
    wout = nc.dram_tensor("wout", [P, 2 * E], bf16, kind="ExternalInput")
    out = nc.dram_tensor("out", [TPC, E], bf16, kind="ExternalOutput")

    with TileContext(nc) as tc:
        with (
            tc.tile_pool(name="const", bufs=1) as cpool,
            tc.tile_pool(name="gath", bufs=5) as gpool,
            tc.tile_pool(name="gtail", bufs=1) as gtpool,
            tc.tile_pool(name="mask", bufs=8) as mpool,
            tc.tile_pool(name="spill", bufs=2) as sppool,
            tc.tile_pool(name="spmask", bufs=2) as smpool,
            tc.tile_pool(name="y", bufs=8) as ypool,
            tc.tile_pool(name="o", bufs=1) as opool,
            tc.tile_pool(name="psy", bufs=1, space="PSUM") as psy,
            tc.tile_pool(name="pso", bufs=4, space="PSUM") as pso,
        ):
            # idxa feeds the very first gather: load it first on the HWDGE
            # path (SP) so the first SWDGE desc-gen can start ~1.5us in.
            # load order matters: HWDGE desc-gens serialize at 625ns each and
            # transfers queue behind the first gather, so the tensors that
            # gate desc-gen (idxa/idxsp/idxb) and the mask inputs
            # (io64/tokloc/wslot) go first; the rest can land mid-stream.
            idxa_sb = cpool.tile([P, ACOLS], i16)
            nc.sync.dma_start(out=idxa_sb[:], in_=idxa[:, :])
            idxsp_sb = cpool.tile([P, SPILL_TOT], i32)
            nc.sync.dma_start(out=idxsp_sb[:], in_=idxsp[:, :])
            idxb_sb = cpool.tile([P, TOTCOLS - ACOLS], i16)
            nc.sync.dma_start(out=idxb_sb[:], in_=idxb[:, :])
            io64_sb = cpool.tile([P, WTOK], i16)
            nc.sync.dma_start(out=io64_sb[:], in_=iota64[:, :])
            tl_sb = cpool.tile([P, MAIN_SLABS], i16)
            nc.sync.dma_start(out=tl_sb[:], in_=tokloc[:, :])
            w_sb = cpool.tile([P, MAIN_SLABS], bf16)
            nc.sync.dma_start(out=w_sb[:], in_=wslot[:, :])
            io512_sb = cpool.tile([P, HTOK], i16)
            nc.sync.dma_start(out=io512_sb[:], in_=iota512[:, :])
            wsp_sb = cpool.tile([P, SPILL_TOT], bf16)
            nc.sync.dma_start(out=wsp_sb[:], in_=wsp[:, :])
            tsp_sb = cpool.tile([P, SPILL_TOT], i16)
            nc.sync.dma_start(out=tsp_sb[:], in_=toksp[:, :])
            wo_sb = cpool.tile([P, 2 * E], bf16)
            nc.sync.dma_start(out=wo_sb[:], in_=wout[:, :])

            nc.gpsimd.load_library(library_config.mlp)

            # yT psum banks, one per (half, pair of 128-token groups);
            # col = g_loc*2P + ch*P + window.
            ytp = {}
            for hh in range(HALVES):
                for pr in range(2):
                    t_ = psy.tile([P, 4 * P], fp32, tag=f"yt{hh}{pr}")
                    nc.vector.memset(t_[:], 0.0)
                    ytp[hh, pr] = t_

            def idx_cols(lo, hi):
                """idx AP for global idx16 column range [lo, hi)."""
                if hi <= ACOLS:
                    return idxa_sb[:, lo:hi]
                return idxb_sb[:, lo - ACOLS:hi - ACOLS]

            def spill_dma(hh, s):
                col = hh * SPILL_SLABS + s
                nc.gpsimd.indirect_dma_start(
                    out=sp_tiles[hh][:, s, :],
                    out_offset=None,
                    in_=kb[:, :],
                    in_offset=bass.IndirectOffsetOnAxis(
                        ap=idxsp_sb[:, col:col + 1], axis=0
                    ),
                )

            sp_tiles = []
            for hh in range(HALVES):
                sp = sppool.tile([P, SPILL_SLABS, D], bf16, tag=f"sp{hh}")
                sp_tiles.append(sp)

            # gather calls in h0-first order so h0 finishes 8 calls early and
            # its whole epilogue overlaps h1's stream; the (h1,b7) call is
            # split so the tail epilogue starts after a short transfer.
            # Spill desc-gens (1038ns each, vs only ~114ns Pool slack per
            # 1342ns call desc-gen) are interleaved where their stall hurts
            # least; each call's column base is (h*NB+b)*COLS.
            gtiles = {}

            def gather_call(hh, b):
                base = (hh * NB + b) * COLS
                g = gpool.tile([P, NW, D], bf16, tag="g")
                nc.gpsimd.dma_gather(
                    out_ap=g[:],
                    in_ap=kb[b * CHUNK:(b + 1) * CHUNK, :],
                    idxs_ap=idx_cols(base, base + COLS),
                    num_idxs=NIDX_CALL,
                    num_idxs_reg=NIDX_CALL,
                    elem_size=D,
                )
                gtiles[hh, b] = g

            gt = []
            for b in range(NB):
                gather_call(0, b)
                if b >= 5:
                    spill_dma(0, b - 5)
            for b in range(NB - 1):
                gather_call(1, b)
                if b in (2, 4, 6):
                    spill_dma(1, (b - 2) // 2)
            tbase = (NB + NB - 1) * COLS + 0 * COLS
            sub = NIDX_CALL // TAILSPLIT
            subc = COLS // TAILSPLIT
            tbase = (1 * NB + NB - 1) * COLS
            for q in range(TAILSPLIT):
                g = gtpool.tile([P, NW // TAILSPLIT, D], bf16, tag=f"t{q}")
                nc.gpsimd.dma_gather(
                    out_ap=g[:],
                    in_ap=kb[(NB - 1) * CHUNK:NB * CHUNK, :],
                    idxs_ap=idx_cols(tbase + q * subc, tbase + (q + 1) * subc),
                    num_idxs=sub,
                    num_idxs_reg=sub,
                    elem_size=D,
                )
                gt.append(g)

            def lhs_slab(hh, b, j, ch):
                if hh == 1 and b == NB - 1:
                    per = NW // TAILSPLIT
                    return gt[j // per][:, j % per, ch * P:(ch + 1) * P]
                return gtiles[hh, b][:, j, ch * P:(ch + 1) * P]

            def build_mask(hh, b, tag, eng=None):
                blk = (b * HALVES + hh) * NW
                mask = mpool.tile([P, NW, WTOK], bf16, tag=tag)
                eng = eng or nc.vector
                eng.tensor_tensor(
                    out=mask[:],
                    in0=io64_sb[:].unsqueeze(1).broadcast_to([P, NW, WTOK]),
                    in1=tl_sb[:, blk:blk + NW].unsqueeze(2)
                        .broadcast_to([P, NW, WTOK]),
                    op=eq,
                )
                eng.tensor_tensor(
                    out=mask[:],
                    in0=mask[:],
                    in1=w_sb[:, blk:blk + NW].unsqueeze(2)
                        .broadcast_to([P, NW, WTOK]),
                    op=mul,
                )
                return mask

            def do_bucket(hh, b, eng=None):
                mask = build_mask(hh, b, "m", eng=eng)
                for j in range(NW):
                    for ch in range(2):
                        pr = j // 4
                        col = ((j // 2) % 2) * 2 * P + ch * P + (j % 2) * WTOK
                        nc.tensor.matmul(
                            out=ytp[hh, pr][:, col:col + WTOK],
                            lhsT=lhs_slab(hh, b, j, ch),
                            rhs=mask[:, j, :],
                            start=False,
                            stop=(hh == 1 and b == NB - 1
                                  and j % 4 == 3 and ch == 1),
                            skip_group_check=True,
                        )

            # spill masks depend only on host constants; they are built
            # mid-loop (after buckets 1/2) so DVE's in-order queue reaches
            # them well before the spill matmuls need them, without pushing
            # the per-bucket masks past their data arrival.
            spill_masks = {}

            def build_spill_mask(hh):
                msp = smpool.tile([P, SPILL_SLABS, HTOK], bf16, tag=f"msp{hh}")
                sblk = hh * SPILL_SLABS
                nc.vector.tensor_tensor(
                    out=msp[:],
                    in0=io512_sb[:].unsqueeze(1)
                        .broadcast_to([P, SPILL_SLABS, HTOK]),
                    in1=tsp_sb[:, sblk:sblk + SPILL_SLABS].unsqueeze(2)
                        .broadcast_to([P, SPILL_SLABS, HTOK]),
                    op=eq,
                )
                nc.vector.tensor_tensor(
                    out=msp[:],
                    in0=msp[:],
                    in1=wsp_sb[:, sblk:sblk + SPILL_SLABS].unsqueeze(2)
                        .broadcast_to([P, SPILL_SLABS, HTOK]),
                    op=mul,
                )
                spill_masks[hh] = msp

            def do_spill(hh, last=False):
                msp = spill_masks[hh]
                for s in range(SPILL_SLABS):
                    for ch in range(2):
                        for pr in range(2):
                            for g in range(2):
                                col = g * 2 * P + ch * P
                                nc.tensor.matmul(
                                    out=ytp[hh, pr][:, col:col + P],
                                    lhsT=sp_tiles[hh][:, s,
                                                      ch * P:(ch + 1) * P],
                                    rhs=msp[:, s,
                                            (pr * 2 + g) * P:
                                            (pr * 2 + g + 1) * P],
                                    start=False,
                                    stop=(last and s == SPILL_SLABS - 1),
                                    skip_group_check=True,
                                )

            def copy_eng(eng, out_ap, in_ap):
                if eng == "act":
                    nc.scalar.copy(out=out_ap, in_=in_ap)
                else:
                    nc.vector.tensor_copy(out=out_ap, in_=in_ap)

            def emit_ybs(hh, yb_engines, groups):
                ybs = {}
                for g4 in groups:
                    yb = ypool.tile([P, 2 * P], bf16, tag=f"yb{hh}{g4}")
                    src = ytp[hh, g4 // 2][:, (g4 % 2) * 2 * P:
                                           (g4 % 2 + 1) * 2 * P]
                    copy_eng(yb_engines[g4 % len(yb_engines)], yb[:], src)
                    ybs[g4] = yb
                return ybs

            def emit_stage2(hh, pair, ybs, osb_engines):
                # stores are paired (two 128-token groups per dma_start):
                # HWDGE desc-gens cost 625ns each, run after the store's sem
                # wait, and serialize -- fewer, bigger stores shorten the tail
                ot = opool.tile([P, 2, E], bf16, tag=f"os{hh}{pair}")
                for gi in range(2):
                    g4 = pair * 2 + gi
                    ops = pso.tile([P, E], fp32, tag="ops")
                    for ch in range(2):
                        nc.tensor.matmul(
                            out=ops[:],
                            lhsT=ybs[g4][:, ch * P:(ch + 1) * P],
                            rhs=wo_sb[:, ch * E:(ch + 1) * E],
                            start=(ch == 0),
                            stop=(ch == 1),
                        )
                    copy_eng(osb_engines[g4 % len(osb_engines)],
                             ot[:, gi, :], ops[:])
                row0 = (hh * 4 + pair * 2) * P
                nc.sync.dma_start(
                    out=out[row0:row0 + 2 * P, :]
                        .rearrange("(c p) e -> p c e", c=2),
                    in_=ot[:],
                )

            def epilogue(hh, yb_engines, osb_engines):
                ybs = emit_ybs(hh, yb_engines, range(HTOK // P))
                for pair in range(2):
                    emit_stage2(hh, pair, ybs, osb_engines)

            # h0 phase: buckets, spill matmuls last (carrying the bank
            # stops), then the whole h0 epilogue -- all mid-stream while h1's
            # gathers transfer.  DVE owns masks, so h0's copies go to Act.
            for b in range(NB):
                do_bucket(0, b)
                if b == 1:
                    build_spill_mask(0)
            do_spill(0, last=True)
            ybs0 = emit_ybs(0, ["act"], range(HTOK // P))
            emit_stage2(0, 0, ybs0, ["act"])
            emit_stage2(0, 1, ybs0, ["act"])

            for b in range(NB - 1):
                do_bucket(1, b)
                if b == 0:
                    build_spill_mask(1)
            do_spill(1)
            do_bucket(1, NB - 1)
            epilogue(1, ["dve", "act"], ["act", "dve"])

    nc.compile()
    return nc


def _host_prep(weights, indexes, w_out):
    """Bucket/sort (token,k) pairs per core and build all device-side arrays."""
    import ml_dtypes

    bfloat16 = ml_dtypes.bfloat16
    wflat = np.ascontiguousarray(weights, dtype=np.float32).reshape(NTOK, K)
    iflat = np.ascontiguousarray(indexes).reshape(NTOK, K).astype(np.int64)

    woutT = np.ascontiguousarray(w_out, dtype=np.float32).T      # [D, E]
    wout_host = np.ascontiguousarray(
        woutT.reshape(2, P, E).transpose(1, 0, 2).reshape(P, 2 * E)
    ).astype(bfloat16)
    iota64_h = np.ascontiguousarray(
        np.broadcast_to(np.arange(WTOK, dtype=np.int16), (P, WTOK))
    )
    iota512_h = np.ascontiguousarray(
        np.broadcast_to(np.arange(HTOK, dtype=np.int16), (P, HTOK))
    )

    in_maps = []
    for c in range(NCORES):
        ic = iflat[c * TPC:(c + 1) * TPC].ravel()          # [16384]
        wc = wflat[c * TPC:(c + 1) * TPC].ravel()
        t = np.repeat(np.arange(TPC, dtype=np.int64), K)   # token per pair

        h = t // HTOK
        wsub = (t % HTOK) // WTOK
        b = ic // CHUNK
        key = (b * HALVES + h) * NW + wsub                 # 0..127 slab id

        order = np.argsort(key, kind="stable")
        ks = key[order]
        iv = ic[order]
        wv = wc[order]
        tv = t[order]
        starts = np.searchsorted(ks, np.arange(MAIN_SLABS))
        rank = np.arange(TPC * K) - starts[ks]

        idx16_host = np.zeros((P, TOTCOLS), np.int16)
        wslot_host = np.zeros((P, MAIN_SLABS), np.float32)
        tokloc_host = np.zeros((P, MAIN_SLABS), np.int16)
        idxsp_host = np.zeros((P, SPILL_TOT), np.int32)
        wsp_host = np.zeros((P, SPILL_TOT), np.float32)
        toksp_host = np.zeros((P, SPILL_TOT), np.int16)

        main = rank < P
        mk, mr = ks[main], rank[main]
        mi, mw, mt = iv[main], wv[main], tv[main]
        mb = mk // (HALVES * NW)
        mh = (mk // NW) % HALVES
        mj = mk % NW
        # slot within the call, and the call's global idx16 column base
        slot = mj * P + mr
        col = (mh * NB + mb) * COLS + slot // 16
        idx_local = (mi - mb * CHUNK).astype(np.int16)
        idx16_host[slot % 16, col] = idx_local             # interp layout
        idx16_host[16 + slot % 16, col] = idx_local        # NEFF Q7 layout
        wslot_host[mr, mk] = mw
        tokloc_host[mr, mk] = (mt - (mh * HTOK + mj * WTOK)).astype(np.int16)

        sh = (ks[~main] // NW) % HALVES                    # spill half
        si, sw, st = iv[~main], wv[~main], tv[~main]
        for hh in range(HALVES):
            sel = sh == hh
            n = int(sel.sum())
            if n > SPILL_CAP:
                raise ValueError(
                    f"spill overflow: core {c} half {hh} needs {n} > {SPILL_CAP}"
                )
            r = np.arange(n)
            idxsp_host[r % P, hh * SPILL_SLABS + r // P] = si[sel]
            wsp_host[r % P, hh * SPILL_SLABS + r // P] = sw[sel]
            toksp_host[r % P, hh * SPILL_SLABS + r // P] = (
                st[sel] - hh * HTOK
            ).astype(np.int16)

        in_maps.append({
            "idxa": np.ascontiguousarray(idx16_host[:, :ACOLS]),
            "idxb": np.ascontiguousarray(idx16_host[:, ACOLS:]),
            "idxsp": idxsp_host,
            "wslot": wslot_host.astype(bfloat16),
            "tokloc": tokloc_host,
            "wsp": wsp_host.astype(bfloat16),
            "toksp": toksp_host,
            "iota64": iota64_h,
            "iota512": iota512_h,
            "wout": wout_host,
        })
    return in_maps


def kernel(weights, indexes, knowledge_base, w_out, b_out):
    import ml_dtypes
    from concourse.bass_utils import run_bass_kernel_spmd

    if "nc" not in _CACHE:
        _CACHE["nc"] = _build_bass()
    nc = _CACHE["nc"]

    kb_host = np.ascontiguousarray(knowledge_base, dtype=np.float32).astype(
        ml_dtypes.bfloat16
    )
    in_maps = _host_prep(weights, indexes, w_out)
    for m in in_maps:
        m["kb"] = kb_host

    res = run_bass_kernel_spmd(nc, in_maps, list(range(NCORES)))
    out = np.concatenate(
        [res.results[c]["out"].astype(np.float32) for c in range(NCORES)],
        axis=0,
    )
    out += np.asarray(b_out, dtype=np.float32)[None, :]
    return out.reshape(B, T, E)


# revision 47
# speedup vs baseline: 1.6993x; 1.0788x over previous
"""Trainium2 Bass kernel for nn_KnowledgeBaseLookup (bucketed dma_gather design).

Computation (see reference):
    lookup = knowledge_base[indexes]            # (B,T,K,D) gather
    y      = einsum('btk,btkd->btd', weights, lookup)
    out    = y @ w_out.T + b_out                # (B,T,E)

Sharding: data-parallel over the B*T token dim across 8 cores; the
knowledge_base table is replicated per core (converted to bf16 and row-
permuted on host — host prep also does the index bucketing/sorting).

Per-core design (1024 tokens, 16384 gathered rows), all-bf16 datapath:
  Rows are gathered with the batched `dma_gather` custom op (1024 indices
  per call, the Q7 ucode's limit) from a bf16 copy of the table, at the
  price of int16 indices: indices are bucketed by table chunk of 32768 rows
  so chunk-local indices fit in int16, with the chunk base carried by the
  in_ap view.

  Layout: tokens split into 2 halves of 512; each half into 8 subgroups of
  64 tokens.  Pairs are bucketed by (chunk b, half h, subgroup j) into 128
  slabs of exactly 128 slots: the host applies a per-core BIJECTIVE row
  permutation to the table, chosen so that every (chunk, half, subgroup)
  bucket receives exactly its 128-slot capacity (a balanced-assignment
  problem over the ~3% of rows used by more than one subgroup, then exact
  fill with single-use rows).  This removes the overflow-spill path (and
  its Pool desc-gens / DVE masks / PE matmuls) and all slot padding.

  Calls run h0-first so h0's whole epilogue (PSUM->SBUF copies, out_proj,
  stores) overlaps h1's gather stream; the last call (h1, chunk 7) is
  issued as two 512-row calls so the tail epilogue starts after a short
  transfer.

  Reduction: for each slab, a [128,64] bf16 mask M[slot, j] =
  w[slot] * (tokloc[slot] == j) is built on DVE (is_equal on an iota table,
  then multiply; tokloc/weights are host-prepped per slot).  PE matmuls
  lhsT=rows (bf16) x rhs=mask accumulate yT[d, token] into per-(half,pair)
  PSUM banks (zeroed once via memset).  Stage 2 (out_proj) contracts yT
  (bf16) with w_out.T (bf16) per 128-token group (bias is added on host),
  copies PSUM->SBUF in bf16 split across the Activation and DVE engines,
  and stores two 128-token groups per dma_start (HWDGE desc-gens serialize
  at 625ns); the host upcasts to fp32.

  The dma_gather Q7 ucode reads index i of a call from the idx tile at
  [16 + i%16, i//16] on the NEFF path (queue 0 channel base), while the
  bass-level interpreter reads [i%16, i//16]; the host writes both bands.
"""

import numpy as np

B, T, K = 4, 2048, 16
C, D, E = 262144, 256, 512
NCORES = 8
NTOK = B * T                      # 8192 tokens
TPC = NTOK // NCORES              # 1024 tokens per core
P = 128
HALVES = 2
HTOK = TPC // HALVES              # 512 tokens per half
NB = 8                            # value chunks
CHUNK = C // NB                   # 32768 rows, int16-addressable
NW = 8                            # subgroups per half
WTOK = HTOK // NW                 # 64 tokens per subgroup
NIDX_CALL = NW * P                # 1024 indices per per-half call
NGROUPS = HALVES * NW             # 16 (half, subgroup) groups
MAIN_SLABS = NB * NGROUPS         # 128
COLS = NIDX_CALL // 16            # 64 idx16 columns per per-half call
TAILSPLIT = 2                     # last call issued as 2x512-row gathers
TOTCOLS = HALVES * NB * COLS      # 1024
ACOLS = 2 * COLS                  # calls 0-1 = first 128 columns

_CACHE = {}


def _build_bass():
    import concourse.mybir as mybir
    from concourse import bacc, library_config
    from concourse.tile import TileContext

    fp32 = mybir.dt.float32
    bf16 = mybir.dt.bfloat16
    i16 = mybir.dt.int16
    eq = mybir.AluOpType.is_equal
    mul = mybir.AluOpType.mult
    nc = bacc.Bacc(
        "TRN2", target_bir_lowering=False, debug=False, num_devices=NCORES,
        dynamic_dma_scratch_size=49152,
    )

    kb = nc.dram_tensor("kb", [C, D], bf16, kind="ExternalInput")
    idxa = nc.dram_tensor("idxa", [P, ACOLS], i16, kind="ExternalInput")
    idxb = nc.dram_tensor("idxb", [P, TOTCOLS - ACOLS], i16,
                          kind="ExternalInput")
    wslot = nc.dram_tensor("wslot", [P, MAIN_SLABS], bf16, kind="ExternalInput")
    tokloc = nc.dram_tensor("tokloc", [P, MAIN_SLABS], i16, kind="ExternalInput")
    iota64 = nc.dram_tensor("iota64", [P, WTOK], i16, kind="ExternalInput")
    wout = nc.dram_tensor("wout", [P, 2 * E], bf16, kind="ExternalInput")
    out = nc.dram_tensor("out", [TPC, E], bf16, kind="ExternalOutput")

    with TileContext(nc) as tc:
        with (
            tc.tile_pool(name="const", bufs=1) as cpool,
            tc.tile_pool(name="gath", bufs=5) as gpool,
            tc.tile_pool(name="gtail", bufs=1) as gtpool,
            tc.tile_pool(name="mask", bufs=8) as mpool,
            tc.tile_pool(name="y", bufs=8) as ypool,
            tc.tile_pool(name="o", bufs=1) as opool,
            tc.tile_pool(name="psy", bufs=1, space="PSUM") as psy,
            tc.tile_pool(name="pso", bufs=4, space="PSUM") as pso,
        ):
            # load order matters: HWDGE desc-gens serialize at 625ns each and
            # transfers queue behind the first gather, so the tensors that
            # gate desc-gen (idxa/idxb) and the mask inputs (io64/tokloc/
            # wslot) go first; wout can land mid-stream.
            idxa_sb = cpool.tile([P, ACOLS], i16)
            nc.sync.dma_start(out=idxa_sb[:], in_=idxa[:, :])
            idxb_sb = cpool.tile([P, TOTCOLS - ACOLS], i16)
            nc.sync.dma_start(out=idxb_sb[:], in_=idxb[:, :])
            io64_sb = cpool.tile([P, WTOK], i16)
            nc.sync.dma_start(out=io64_sb[:], in_=iota64[:, :])
            tl_sb = cpool.tile([P, MAIN_SLABS], i16)
            nc.sync.dma_start(out=tl_sb[:], in_=tokloc[:, :])
            w_sb = cpool.tile([P, MAIN_SLABS], bf16)
            nc.sync.dma_start(out=w_sb[:], in_=wslot[:, :])
            wo_sb = cpool.tile([P, 2 * E], bf16)
            nc.sync.dma_start(out=wo_sb[:], in_=wout[:, :])

            nc.gpsimd.load_library(library_config.mlp)

            # yT psum banks, one per (half, pair of 128-token groups);
            # col = g_loc*2P + ch*P + window.
            ytp = {}
            for hh in range(HALVES):
                for pr in range(2):
                    t_ = psy.tile([P, 4 * P], fp32, tag=f"yt{hh}{pr}")
                    nc.vector.memset(t_[:], 0.0)
                    ytp[hh, pr] = t_

            def idx_cols(lo, hi):
                """idx AP for global idx16 column range [lo, hi)."""
                if hi <= ACOLS:
                    return idxa_sb[:, lo:hi]
                return idxb_sb[:, lo - ACOLS:hi - ACOLS]

            # gather calls in h0-first order so h0 finishes 8 calls early and
            # its whole epilogue overlaps h1's stream; the (h1,b7) call is
            # split so the tail epilogue starts after a short transfer.
            # Each call's column base is (h*NB+b)*COLS.
            gtiles = {}

            def gather_call(hh, b):
                base = (hh * NB + b) * COLS
                g = gpool.tile([P, NW, D], bf16, tag="g")
                nc.gpsimd.dma_gather(
                    out_ap=g[:],
                    in_ap=kb[b * CHUNK:(b + 1) * CHUNK, :],
                    idxs_ap=idx_cols(base, base + COLS),
                    num_idxs=NIDX_CALL,
                    num_idxs_reg=NIDX_CALL,
                    elem_size=D,
                )
                gtiles[hh, b] = g

            for b in range(NB):
                gather_call(0, b)
            for b in range(NB - 1):
                gather_call(1, b)
            gt = []
            tbase = (1 * NB + NB - 1) * COLS
            sub = NIDX_CALL // TAILSPLIT
            subc = COLS // TAILSPLIT
            for q in range(TAILSPLIT):
                g = gtpool.tile([P, NW // TAILSPLIT, D], bf16, tag=f"t{q}")
                nc.gpsimd.dma_gather(
                    out_ap=g[:],
                    in_ap=kb[(NB - 1) * CHUNK:NB * CHUNK, :],
                    idxs_ap=idx_cols(tbase + q * subc, tbase + (q + 1) * subc),
                    num_idxs=sub,
                    num_idxs_reg=sub,
                    elem_size=D,
                )
                gt.append(g)

            def lhs_slab(hh, b, j, ch):
                if hh == 1 and b == NB - 1:
                    per = NW // TAILSPLIT
                    return gt[j // per][:, j % per, ch * P:(ch + 1) * P]
                return gtiles[hh, b][:, j, ch * P:(ch + 1) * P]

            def build_mask(hh, b, tag, eng=None):
                blk = (b * HALVES + hh) * NW
                mask = mpool.tile([P, NW, WTOK], bf16, tag=tag)
                eng = eng or nc.vector
                eng.tensor_tensor(
                    out=mask[:],
                    in0=io64_sb[:].unsqueeze(1).broadcast_to([P, NW, WTOK]),
                    in1=tl_sb[:, blk:blk + NW].unsqueeze(2)
                        .broadcast_to([P, NW, WTOK]),
                    op=eq,
                )
                eng.tensor_tensor(
                    out=mask[:],
                    in0=mask[:],
                    in1=w_sb[:, blk:blk + NW].unsqueeze(2)
                        .broadcast_to([P, NW, WTOK]),
                    op=mul,
                )
                return mask

            def do_bucket(hh, b, eng=None):
                mask = build_mask(hh, b, "m", eng=eng)
                for j in range(NW):
                    for ch in range(2):
                        pr = j // 4
                        col = ((j // 2) % 2) * 2 * P + ch * P + (j % 2) * WTOK
                        nc.tensor.matmul(
                            out=ytp[hh, pr][:, col:col + WTOK],
                            lhsT=lhs_slab(hh, b, j, ch),
                            rhs=mask[:, j, :],
                            start=False,
                            stop=(b == NB - 1 and j % 4 == 3 and ch == 1),
                            skip_group_check=True,
                        )

            def copy_eng(eng, out_ap, in_ap):
                if eng == "act":
                    nc.scalar.copy(out=out_ap, in_=in_ap)
                else:
                    nc.vector.tensor_copy(out=out_ap, in_=in_ap)

            def emit_ybs(hh, yb_engines, groups):
                ybs = {}
                for g4 in groups:
                    yb = ypool.tile([P, 2 * P], bf16, tag=f"yb{hh}{g4}")
                    src = ytp[hh, g4 // 2][:, (g4 % 2) * 2 * P:
                                           (g4 % 2 + 1) * 2 * P]
                    copy_eng(yb_engines[g4 % len(yb_engines)], yb[:], src)
                    ybs[g4] = yb
                return ybs

            def emit_stage2(hh, pair, ybs, osb_engines):
                # stores are paired (two 128-token groups per dma_start):
                # HWDGE desc-gens cost 625ns each, run after the store's sem
                # wait, and serialize -- fewer, bigger stores shorten the tail
                ot = opool.tile([P, 2, E], bf16, tag=f"os{hh}{pair}")
                for gi in range(2):
                    g4 = pair * 2 + gi
                    ops = pso.tile([P, E], fp32, tag="ops")
                    for ch in range(2):
                        nc.tensor.matmul(
                            out=ops[:],
                            lhsT=ybs[g4][:, ch * P:(ch + 1) * P],
                            rhs=wo_sb[:, ch * E:(ch + 1) * E],
                            start=(ch == 0),
                            stop=(ch == 1),
                        )
                    copy_eng(osb_engines[g4 % len(osb_engines)],
                             ot[:, gi, :], ops[:])
                row0 = (hh * 4 + pair * 2) * P
                nc.sync.dma_start(
                    out=out[row0:row0 + 2 * P, :]
                        .rearrange("(c p) e -> p c e", c=2),
                    in_=ot[:],
                )

            def epilogue(hh, yb_engines, osb_engines):
                ybs = emit_ybs(hh, yb_engines, range(HTOK // P))
                for pair in range(2):
                    emit_stage2(hh, pair, ybs, osb_engines)

            # h0 phase: buckets then the whole h0 epilogue -- all mid-stream
            # while h1's gathers transfer.  DVE owns masks, so h0's copies go
            # to Act.
            for b in range(NB):
                do_bucket(0, b)
            epilogue(0, ["act"], ["act"])

            for b in range(NB):
                do_bucket(1, b)
            epilogue(1, ["act", "dve"], ["dve", "act"])

    nc.compile()
    return nc


def _balance_chunks(rows, groups):
    """Assign each distinct table row to a chunk so every (group, chunk)
    bucket gets exactly P pairs.  rows/groups: per-pair arrays [TPC*K].
    Multi-use rows (all pairs of one row must share a chunk) are placed
    greedily; single-use rows then fill every bucket to exactly P.
    Returns (chunk_of_pair, uniq_rows, chunk_of_uniq)."""
    order = np.argsort(rows, kind="stable")
    rs, gs = rows[order], groups[order]
    uq, st = np.unique(rs, return_index=True)
    en = np.append(st[1:], len(rs))
    load = np.zeros((NGROUPS, NB), np.int64)
    chunk_of_pair = np.empty(len(rows), np.int64)
    chunk_of_uniq = np.empty(len(uq), np.int64)

    multi = np.where((en - st) > 1)[0]
    for i in multi:
        g_counts = np.bincount(gs[st[i]:en[i]], minlength=NGROUPS)
        ok = ((load + g_counts[:, None]) <= P).all(axis=0)
        cand = np.where(ok)[0]
        if len(cand) == 0:
            raise ValueError("balanced chunk assignment infeasible")
        c = cand[np.argmin(load[:, cand].sum(axis=0))]
        load[:, c] += g_counts
        chunk_of_pair[order[st[i]:en[i]]] = c
        chunk_of_uniq[i] = c

    singles = np.where((en - st) == 1)[0]
    sg = gs[st[singles]]
    for g in range(NGROUPS):
        gi = singles[sg == g]
        need = P - load[g]
        if need.sum() != len(gi) or (need < 0).any():
            raise ValueError("balanced chunk assignment infeasible")
        cs = np.repeat(np.arange(NB), need)
        chunk_of_pair[order[st[gi]]] = cs
        chunk_of_uniq[gi] = cs
        load[g] += np.bincount(cs, minlength=NB)
    assert (load == P).all()
    return chunk_of_pair, uq, chunk_of_uniq


def _host_prep(weights, indexes, knowledge_base, w_out):
    """Permute/bucket per core and build all device-side arrays."""
    import ml_dtypes

    bfloat16 = ml_dtypes.bfloat16
    wflat = np.ascontiguousarray(weights, dtype=np.float32).reshape(NTOK, K)
    iflat = np.ascontiguousarray(indexes).reshape(NTOK, K).astype(np.int64)
    kb_bf = np.ascontiguousarray(knowledge_base, dtype=np.float32).astype(
        bfloat16
    )

    woutT = np.ascontiguousarray(w_out, dtype=np.float32).T      # [D, E]
    wout_host = np.ascontiguousarray(
        woutT.reshape(2, P, E).transpose(1, 0, 2).reshape(P, 2 * E)
    ).astype(bfloat16)
    iota64_h = np.ascontiguousarray(
        np.broadcast_to(np.arange(WTOK, dtype=np.int16), (P, WTOK))
    )

    in_maps = []
    for c in range(NCORES):
        ic = iflat[c * TPC:(c + 1) * TPC].ravel()          # [16384]
        wc = wflat[c * TPC:(c + 1) * TPC].ravel()
        t = np.repeat(np.arange(TPC, dtype=np.int64), K)   # token per pair

        h = t // HTOK
        wsub = (t % HTOK) // WTOK
        grp = h * NW + wsub                                # 0..15

        b, uq, cuq = _balance_chunks(ic, grp)

        # per-core permuted table: referenced rows get the low slots of
        # their assigned chunk; unreferenced rows fill the rest.  newpos is
        # a bijection, so the device still gathers from the full table.
        newpos = np.full(C, -1, np.int64)
        used = np.zeros(NB, np.int64)
        for cc in range(NB):
            rows_c = uq[cuq == cc]
            newpos[rows_c] = cc * CHUNK + np.arange(len(rows_c))
            used[cc] = len(rows_c)
        free = np.where(newpos < 0)[0]
        slots = [cc * CHUNK + np.arange(used[cc], CHUNK) for cc in range(NB)]
        newpos[free] = np.concatenate(slots)
        kb_perm = np.empty_like(kb_bf)
        kb_perm[newpos] = kb_bf

        key = (b * HALVES + h) * NW + wsub                 # 0..127 slab id
        order = np.argsort(key, kind="stable")
        ks = key[order]
        iv = newpos[ic[order]]                             # permuted row ids
        wv = wc[order]
        tv = t[order]
        starts = np.searchsorted(ks, np.arange(MAIN_SLABS))
        rank = np.arange(TPC * K) - starts[ks]
        if rank.max() >= P:
            raise ValueError("bucket overflow after balancing")

        idx16_host = np.zeros((P, TOTCOLS), np.int16)
        wslot_host = np.zeros((P, MAIN_SLABS), np.float32)
        tokloc_host = np.zeros((P, MAIN_SLABS), np.int16)

        mk, mr = ks, rank
        mb = mk // (HALVES * NW)
        mh = (mk // NW) % HALVES
        mj = mk % NW
        slot = mj * P + mr
        col = (mh * NB + mb) * COLS + slot // 16
        idx_local = (iv - mb * CHUNK).astype(np.int16)
        idx16_host[slot % 16, col] = idx_local             # interp layout
        idx16_host[16 + slot % 16, col] = idx_local        # NEFF Q7 layout
        wslot_host[mr, mk] = wv
        tokloc_host[mr, mk] = (tv - (mh * HTOK + mj * WTOK)).astype(np.int16)

        in_maps.append({
            "kb": kb_perm,
            "idxa": np.ascontiguousarray(idx16_host[:, :ACOLS]),
            "idxb": np.ascontiguousarray(idx16_host[:, ACOLS:]),
            "wslot": wslot_host.astype(bfloat16),
            "tokloc": tokloc_host,
            "iota64": iota64_h,
            "wout": wout_host,
        })
    return in_maps


def kernel(weights, indexes, knowledge_base, w_out, b_out):
    from concourse.bass_utils import run_bass_kernel_spmd

    if "nc" not in _CACHE:
        _CACHE["nc"] = _build_bass()
    nc = _CACHE["nc"]

    in_maps = _host_prep(weights, indexes, knowledge_base, w_out)
    res = run_bass_kernel_spmd(nc, in_maps, list(range(NCORES)))
    out = np.concatenate(
        [res.results[c]["out"].astype(np.float32) for c in range(NCORES)],
        axis=0,
    )
    out += np.asarray(b_out, dtype=np.float32)[None, :]
    return out.reshape(B, T, E)
